# revision 1
# baseline (speedup 1.0000x reference)
"""BERT-base encoder (12L, B=4, S=2048, H=768) on 8 Trainium2 NeuronCores.

Sharding: 8 shards of 1024 tokens each (sample b = core//2, seq-half = core%2).
Per layer, each core computes K/V for its own 1024 tokens and AllGathers them
within the core pair owning the sample, so attention sees the full 2048-token
context while all other work stays perfectly data-parallel.

On-chip layout: activations are kept feature-major (h^T, [768, 1024]) so every
projection is a plain accumulated matmul with the stored [in, out] weights as
the stationary operand. Attention computes scores^T ([k, q]) so the
probs @ V matmul needs no transposes; softmax normalization uses a ones-column
appended to V (row 64 of the ctx PSUM accumulates sum(exp)) plus a K=1
ones-matmul to broadcast 1/sumexp across partitions. LayerNorm statistics are
computed with ones-vector matmuls (sums over the partition dim land in PSUM).

Everything after the K/V AllGather is processed in two independent 512-token
q-halves (attention -> O -> LN1 -> FFN -> LN2 per half), which lets the Tile
scheduler overlap the scalar-engine-bound softmax exp of one half with the
PE-bound FFN of the other, and the next layer's K/V projections with the
trailing half's FFN.
"""

import dataclasses

import numpy as np
import ml_dtypes

import concourse.bass as bass
import concourse.tile as tile
from concourse import bacc, mybir
from concourse.bass import IndirectOffsetOnAxis
from concourse.bass_utils import run_bass_kernel_spmd
from concourse.masks import make_identity
from concourse.alu_op_type import AluOpType

F32 = mybir.dt.float32
BF16 = mybir.dt.bfloat16
I32 = mybir.dt.int32
AF = mybir.ActivationFunctionType
BF = ml_dtypes.bfloat16

PHASE_HOOK = None  # optional (tag, layer, nc) callback for profiling builds


def _ph(tag, l, nc):
    if PHASE_HOOK is not None:
        PHASE_HOOK(tag, l, nc)


@dataclasses.dataclass
class Cfg:
    L: int = 12
    NH: int = 12
    DH: int = 64
    FF: int = 3072
    V: int = 30522
    TOK: int = 1024  # tokens per core
    S: int = 2048  # full sequence
    n_cores: int = 8
    EPS: float = 1e-12

    @property
    def Hd(self):
        return self.NH * self.DH

    @property
    def HC(self):
        return self.Hd // 128  # hidden chunks == head pairs

    @property
    def TB(self):
        return self.TOK // 128

    @property
    def KB(self):
        return self.S // 128

    @property
    def FC(self):
        return self.FF // 128

    @property
    def NHALF(self):
        return self.S // self.TOK  # cores per sample

    @property
    def QS(self):
        return [(q, min(512, self.TOK - q)) for q in range(0, self.TOK, 512)]


def _ns(total, size):
    return [(s, min(size, total - s)) for s in range(0, total, size)]


def build(cfg: Cfg, fake_cc: bool = False):
    L, NH, DH, FF, V = cfg.L, cfg.NH, cfg.DH, cfg.FF, cfg.V
    TOK, S, Hd = cfg.TOK, cfg.S, cfg.Hd
    HC, TB, KB, FC, NHALF = cfg.HC, cfg.TB, cfg.KB, cfg.FC, cfg.NHALF
    QS = cfg.QS
    TPH = TOK // 128  # k blocks per gathered half

    nc = bacc.Bacc(
        "TRN2",
        target_bir_lowering=False,
        debug=False,
        enable_asserts=True,
        num_devices=cfg.n_cores,
    )

    # ---------------- DRAM I/O ----------------
    wq_d = nc.dram_tensor("wq", [L, Hd, Hd], BF16, kind="ExternalInput").ap()
    wk_d = nc.dram_tensor("wk", [L, Hd, Hd], BF16, kind="ExternalInput").ap()
    wv_d = nc.dram_tensor("wv", [L, Hd, Hd], BF16, kind="ExternalInput").ap()
    wo_d = nc.dram_tensor("wo", [L, Hd, Hd], BF16, kind="ExternalInput").ap()
    wi_d = nc.dram_tensor("wi", [L, FC, 128, Hd], BF16, kind="ExternalInput").ap()
    wf_d = nc.dram_tensor("wf", [L, HC, 128, FF], BF16, kind="ExternalInput").ap()
    # packed per-layer params: bq|bk|bo|bf|l1w|l1b|l2w|l2b, each HC cols
    par_d = nc.dram_tensor("par", [L, 128, 8 * HC], F32, kind="ExternalInput").ap()
    bi_d = nc.dram_tensor("bi", [L, 128, FC], F32, kind="ExternalInput").ap()
    bv_d = nc.dram_tensor("bv", [L, 1, Hd], BF16, kind="ExternalInput").ap()
    we_d = nc.dram_tensor("wemb", [V, Hd], BF16, kind="ExternalInput").ap()
    pos_d = nc.dram_tensor("pos", [TOK, Hd], F32, kind="ExternalInput").ap()
    # embedding LN gain|bias packed column-wise: [128, w0..w5 b0..b5]
    lne_d = nc.dram_tensor("lne", [128, 2 * HC], F32, kind="ExternalInput").ap()
    ids_d = nc.dram_tensor("ids", [128, TB], I32, kind="ExternalInput").ap()
    mask_d = nc.dram_tensor("mask", [128, KB], F32, kind="ExternalInput").ap()
    y_d = nc.dram_tensor("y", [TOK, Hd], F32, kind="ExternalOutput").ap()

    VW = NH * (DH + 1)  # V row width incl. interleaved ones columns
    KVN = Hd * TOK + TOK * VW
    kv_in = nc.dram_tensor("kv_in", [KVN], BF16, kind="Internal").ap()
    kv_out = nc.dram_tensor("kv_out", [NHALF, KVN], BF16, kind="Internal").ap()
    kvi_K = kv_in[0 : Hd * TOK].rearrange("(h t) -> h t", t=TOK)
    kvi_V = kv_in[Hd * TOK :].rearrange("(t w) -> t w", w=VW)

    groups = [
        [g * NHALF + i for i in range(NHALF)] for g in range(cfg.n_cores // NHALF)
    ]

    with tile.TileContext(nc) as tc:
        # ---------------- persistent SBUF ----------------
        _frees = []  # keep pool-release closures alive for the whole build

        def single(name, shape, dtype):
            t, fr = tc.tile(shape, dtype, name=name)
            _frees.append(fr)
            return t

        h_m = [single(f"h_m{i}", [128, TOK], F32) for i in range(HC)]
        lnin = [single(f"lnin{i}", [128, TOK], F32) for i in range(HC)]
        h_bf = [single(f"h_bf{i}", [128, TOK], BF16) for i in range(HC)]
        qT = [single(f"qT{i}", [128, TOK], BF16) for i in range(HC)]
        kT = [
            [single(f"kT{h}_{i}", [128, TOK], BF16) for i in range(HC)]
            for h in range(NHALF)
        ]
        v_sb = [single(f"v_sb{i}", [128, NH * (DH + 1)], BF16) for i in range(KB)]
        ctxT = [single(f"ctxT{i}", [128, TOK], BF16) for i in range(HC)]
        # half of the FFN activation slots; the rest alias qT/ctxT columns of
        # the half currently in its FFN phase (dead there, live for the other)
        ffx = [single(f"ffx{i}", [128, 512], BF16) for i in range(FC - 2 * HC)]

        ids_sb = single("ids_sb", [128, TB], I32)
        mask_sb = single("mask_sb", [128, KB], F32)
        ones_c = single("ones_c", [128, 1], BF16)  # stats lhsT
        ones_r = single("ones_r", [1, 128], BF16)  # broadcast lhsT
        ident = single("ident", [128, 128], F32)
        eps_sb = single("eps_sb", [128, 1], F32)
        nc.vector.memset(eps_sb[:], 1e-12)

        nc.vector.memset(ones_c[:], 1.0)
        nc.vector.memset(ones_r[:], 1.0)
        make_identity(nc, ident[:])
        nc.sync.dma_start(ids_sb[:], ids_d[:, :])
        nc.sync.dma_start(mask_sb[:], mask_d[:, :])

        def ffT(oc, q0):
            """[128, 512] bf16 slot for FFN activation block oc of the q-half
            starting at column q0."""
            if oc < len(ffx):
                return ffx[oc][:, 0:512]
            oc -= len(ffx)
            if oc < HC:
                return qT[oc][:, q0 : q0 + 512]
            return ctxT[oc - HC][:, q0 : q0 + 512]

        with (
            tc.tile_pool(name="wp", bufs=9) as wp,
            tc.tile_pool(name="wip", bufs=5) as wip,
            tc.tile_pool(name="wfp", bufs=3) as wfp,
            tc.tile_pool(name="expp", bufs=3) as expp,
            tc.tile_pool(name="scr", bufs=2) as scr,
            tc.tile_pool(name="rowf", bufs=2) as rowf,
            tc.tile_pool(name="rowb", bufs=2) as rowb,
            tc.tile_pool(name="stg", bufs=2) as stg,
            tc.tile_pool(name="parp", bufs=2) as parp,
            tc.tile_pool(name="psS", bufs=2, space="PSUM") as psS,  # [128,1024] 2 banks
            tc.tile_pool(name="psC", bufs=2, space="PSUM") as psC,  # [65,512] ctx accum
            tc.tile_pool(name="psW", bufs=2, space="PSUM") as psW,  # [128,512] work
        ):
            # ---------------- embedding ----------------
            _ph('embed', -1, nc)
            lne_t, fr1 = tc.tile([128, 2 * HC], F32, name="lne_t")
            nc.sync.dma_start(lne_t[:], lne_d[:, :])

            bn_sub = 256 if Hd % 256 == 0 else 128
            nsub = Hd // bn_sub
            for tb in range(TB):
                emb_g = stg.tile([128, Hd], BF16, tag="vst", name=f"embg{tb}")
                nc.gpsimd.indirect_dma_start(
                    out=emb_g[:],
                    out_offset=None,
                    in_=we_d[:, :],
                    in_offset=IndirectOffsetOnAxis(ap=ids_sb[:, tb : tb + 1], axis=0),
                )
                pos_t = scr.tile([128, Hd], F32, tag="scr", name=f"pos{tb}")
                nc.sync.dma_start(pos_t[:], pos_d[tb * 128 : (tb + 1) * 128, :])
                x = scr.tile([128, Hd], F32, tag="scr", name=f"embx{tb}")
                nc.vector.tensor_copy(x[:], emb_g[:])
                nc.vector.tensor_add(x[:], x[:], pos_t[:])
                # LN over the free (feature) dim
                stats = scr.tile([128, nsub, 6], F32, tag="bst", name=f"bst{tb}")
                for sgi in range(nsub):
                    nc.vector.bn_stats(
                        stats[:, sgi, :], x[:, sgi * bn_sub : (sgi + 1) * bn_sub]
                    )
                mv = scr.tile([128, 2], F32, tag="bmv", name=f"bmv{tb}")
                nc.vector.bn_aggr(mv[:], stats[:])
                rstd = scr.tile([128, 1], F32, tag="brs", name=f"brs{tb}")
                nc.scalar.activation(rstd[:], mv[:, 1:2], AF.Sqrt, bias=eps_sb[:])
                nc.vector.reciprocal(rstd[:], rstd[:])
                nc.vector.tensor_scalar(
                    x[:], x[:], mv[:, 0:1], rstd[:], AluOpType.subtract, AluOpType.mult
                )
                # transpose to feature-major, then gain/bias per feature chunk
                for hc in range(HC):
                    tpool, ttag = (psS, "sc") if hc % 2 else (psW, "w512")
                    pst = tpool.tile([128, 128], F32, tag=ttag, name=f"tp{tb}_{hc}")
                    nc.tensor.transpose(
                        pst[:], x[:, hc * 128 : (hc + 1) * 128], ident[:]
                    )
                    nc.vector.tensor_scalar(
                        h_m[hc][:, tb * 128 : (tb + 1) * 128],
                        pst[:],
                        lne_t[:, hc : hc + 1],
                        lne_t[:, HC + hc : HC + hc + 1],
                        AluOpType.mult,
                        AluOpType.add,
                    )
                    nc.vector.tensor_copy(
                        h_bf[hc][:, tb * 128 : (tb + 1) * 128],
                        h_m[hc][:, tb * 128 : (tb + 1) * 128],
                    )
            fr1()

            # ---------------- layers ----------------
            for l in range(L):
                par_t = parp.tile([128, 8 * HC], F32, tag="par", name=f"par{l}")
                nc.sync.dma_start(par_t[:], par_d[l])
                bi_t = parp.tile([128, FC], F32, tag="bi", name=f"bi{l}")
                nc.sync.dma_start(bi_t[:], bi_d[l])
                bv_t = parp.tile([1, Hd], BF16, tag="bv", bufs=1, name=f"bv{l}")
                nc.sync.dma_start(bv_t[:], bv_d[l])
                O_BQ, O_BK, O_BO, O_BF = 0, HC, 2 * HC, 3 * HC
                O_L1W, O_L1B, O_L2W, O_L2B = 4 * HC, 5 * HC, 6 * HC, 7 * HC

                # ---- K projection -> kv staging ----
                _ph('K_proj', l, nc)
                wk_t = [
                    wp.tile([128, Hd], BF16, tag="w", name=f"wk{l}_{ic}")
                    for ic in range(HC)
                ]
                for ic in range(HC):
                    nc.sync.dma_start(
                        wk_t[ic][:], wk_d[l, ic * 128 : (ic + 1) * 128, :]
                    )
                for hc in range(HC):
                    ks = stg.tile([128, TOK], BF16, tag="kst", name=f"ks{l}_{hc}")
                    for q0, qn in QS:
                        ps = psC.tile([128, qn], F32, tag="ctx", name=f"psk{l}_{hc}_{q0}")
                        for ic in range(HC):
                            nc.tensor.matmul(
                                ps[:],
                                wk_t[ic][:, hc * 128 : (hc + 1) * 128],
                                h_bf[ic][:, q0 : q0 + qn],
                                start=(ic == 0),
                                stop=(ic == HC - 1),
                            )
                        nc.vector.tensor_scalar_add(
                            ks[:, q0 : q0 + qn], ps[:], par_t[:, O_BK + hc : O_BK + hc + 1]
                        )
                    nc.sync.dma_start(kvi_K[hc * 128 : (hc + 1) * 128, :], ks[:])

                # ---- V projection (token-major, bias via K=1 ones matmul) ----
                _ph('V_proj', l, nc)
                wv_t = [
                    wp.tile([128, Hd], BF16, tag="w", name=f"wv{l}_{ic}")
                    for ic in range(HC)
                ]
                for ic in range(HC):
                    nc.sync.dma_start(
                        wv_t[ic][:], wv_d[l, ic * 128 : (ic + 1) * 128, :]
                    )
                for tb in range(TB):
                    vs = stg.tile([128, VW], BF16, tag="vst", name=f"vs{l}_{tb}")
                    vsv = vs[:].rearrange("p (n e) -> p n e", e=DH + 1)
                    for n0, nn in _ns(Hd, 512):
                        ps = psC.tile([128, nn], F32, tag="ctx", name=f"psv{l}_{tb}_{n0}")
                        nc.tensor.matmul(
                            ps[:],
                            ones_r[0:1, 0:128],
                            bv_t[0:1, n0 : n0 + nn],
                            start=True,
                            stop=False,
                        )
                        for ic in range(HC):
                            nc.tensor.matmul(
                                ps[:],
                                h_bf[ic][:, tb * 128 : (tb + 1) * 128],
                                wv_t[ic][:, n0 : n0 + nn],
                                start=False,
                                stop=(ic == HC - 1),
                            )
                        h0, hn = n0 // DH, nn // DH  # head range of this chunk
                        nc.vector.tensor_copy(
                            vsv[:, h0 : h0 + hn, 0:DH],
                            ps[:].rearrange("p (n d) -> p n d", d=DH),
                        )
                    nc.vector.memset(vsv[:, :, DH : DH + 1], 1.0)
                    nc.sync.dma_start(kvi_V[tb * 128 : (tb + 1) * 128, :], vs[:])

                # ---- AllGather K/V within the sample's core group ----
                _ph('AllGather', l, nc)
                if fake_cc:
                    for half in range(NHALF):
                        nc.sync.dma_start(kv_out[half], kv_in[:])
                else:
                    nc.gpsimd.collective_compute(
                        "AllGather",
                        AluOpType.bypass,
                        replica_groups=groups,
                        ins=[kv_in[:].opt()],
                        outs=[kv_out[:].opt()],
                    )

                # ---- Q projection (overlaps the collective) ----
                _ph('Q_proj', l, nc)
                wq_t = [
                    wp.tile([128, Hd], BF16, tag="w", name=f"wq{l}_{ic}")
                    for ic in range(HC)
                ]
                for ic in range(HC):
                    nc.sync.dma_start(
                        wq_t[ic][:], wq_d[l, ic * 128 : (ic + 1) * 128, :]
                    )
                for hc in range(HC):
                    for q0, qn in QS:
                        ps = psC.tile([128, qn], F32, tag="ctx", name=f"psq{l}_{hc}_{q0}")
                        for ic in range(HC):
                            nc.tensor.matmul(
                                ps[:],
                                wq_t[ic][:, hc * 128 : (hc + 1) * 128],
                                h_bf[ic][:, q0 : q0 + qn],
                                start=(ic == 0),
                                stop=(ic == HC - 1),
                            )
                        nc.vector.tensor_scalar_add(
                            qT[hc][:, q0 : q0 + qn], ps[:],
                            par_t[:, O_BQ + hc : O_BQ + hc + 1],
                        )

                # ---- unpack gathered K/V ----
                _ph('unpack_KV', l, nc)
                for half in range(NHALF):
                    kvo_K = kv_out[half, 0 : Hd * TOK].rearrange("(h t) -> h t", t=TOK)
                    kvo_V = kv_out[half, Hd * TOK :].rearrange("(t w) -> t w", w=VW)
                    for hc in range(HC):
                        nc.sync.dma_start(
                            kT[half][hc][:], kvo_K[hc * 128 : (hc + 1) * 128, :]
                        )
                    for tb in range(TPH):
                        kb = half * TPH + tb
                        nc.sync.dma_start(
                            v_sb[kb][:], kvo_V[tb * 128 : (tb + 1) * 128, :]
                        )

                # ---- per q-half: attention -> O -> LN1 -> FFN -> LN2 ----
                def layer_norm(q0, qn, w_off, b_off, tag):
                    # squares scratch lives in qT columns of this half (dead
                    # between this half's attention and next-layer Q proj)
                    for hc in range(HC):
                        nc.vector.tensor_copy(
                            h_bf[hc][:, q0 : q0 + qn], lnin[hc][:, q0 : q0 + qn]
                        )
                        # squares from the bf16 copy: all-bf16 SBUF operands
                        # let the DVE run in its 2x mode
                        nc.vector.tensor_mul(
                            qT[hc][:, q0 : q0 + qn],
                            h_bf[hc][:, q0 : q0 + qn],
                            h_bf[hc][:, q0 : q0 + qn],
                        )
                    s_ps = psW.tile([1, qn], F32, tag="w512", name=f"sps{tag}{l}_{q0}")
                    q_ps = psW.tile([1, qn], F32, tag="w512", name=f"qps{tag}{l}_{q0}")
                    for hc in range(HC):
                        nc.tensor.matmul(
                            s_ps[:],
                            ones_c[:],
                            h_bf[hc][:, q0 : q0 + qn],
                            start=(hc == 0),
                            stop=(hc == HC - 1),
                        )
                    for hc in range(HC):
                        nc.tensor.matmul(
                            q_ps[:],
                            ones_c[:],
                            qT[hc][:, q0 : q0 + qn],
                            start=(hc == 0),
                            stop=(hc == HC - 1),
                        )
                    t1 = rowf.tile([1, qn], F32, tag="t1", bufs=1, name=f"t1{tag}{l}_{q0}")
                    nc.scalar.square(t1[:], s_ps[:])
                    nc.vector.scalar_tensor_tensor(
                        t1[:],
                        t1[:],
                        -1.0 / Hd,
                        q_ps[:],
                        op0=AluOpType.mult,
                        op1=AluOpType.add,
                    )
                    # t1 = sumsq - sum^2/Hd ; rstd = 1/sqrt(t1/Hd + eps)
                    nc.scalar.activation(
                        t1[:], t1[:], AF.Sqrt, scale=1.0 / Hd, bias=eps_sb[0:1, :]
                    )
                    nc.vector.reciprocal(t1[:], t1[:])
                    mr_b = rowb.tile(
                        [1, 2 * qn], BF16, tag="mrb", bufs=1, name=f"mr{tag}{l}_{q0}"
                    )
                    nc.vector.tensor_scalar_mul(mr_b[:, 0:qn], s_ps[:], 1.0 / Hd)
                    nc.vector.tensor_copy(mr_b[:, qn : 2 * qn], t1[:])
                    bpool, btag = (psS, "sc") if q0 >= 512 else (psW, "w512")
                    m_bc = bpool.tile([128, qn], F32, tag=btag, name=f"mbc{tag}{l}_{q0}")
                    r_bc = psW.tile([128, qn], F32, tag="w512", name=f"rbc{tag}{l}_{q0}")
                    nc.tensor.matmul(
                        m_bc[:], ones_r[0:1, 0:128], mr_b[0:1, 0:qn],
                        start=True, stop=True,
                    )
                    nc.tensor.matmul(
                        r_bc[:], ones_r[0:1, 0:128], mr_b[0:1, qn : 2 * qn],
                        start=True, stop=True,
                    )
                    for hc in range(HC):
                        t = scr.tile([128, qn], F32, tag="scr", name=f"sc{tag}{l}_{q0}_{hc}")
                        nc.vector.tensor_sub(t[:], lnin[hc][:, q0 : q0 + qn], m_bc[:])
                        nc.vector.tensor_mul(t[:], t[:], r_bc[:])
                        nc.vector.tensor_scalar(
                            h_m[hc][:, q0 : q0 + qn],
                            t[:],
                            par_t[:, w_off + hc : w_off + hc + 1],
                            par_t[:, b_off + hc : b_off + hc + 1],
                            AluOpType.mult,
                            AluOpType.add,
                        )
                        nc.vector.tensor_copy(
                            h_bf[hc][:, q0 : q0 + qn], h_m[hc][:, q0 : q0 + qn]
                        )

                for qi, (q0, qn) in enumerate(QS):
                    # ---- attention for this q-half ----
                    _ph(f'attn{qi}', l, nc)
                    for hp in range(HC):
                        ctx = [
                            psC.tile([65, qn], F32, tag="ctx", name=f"ctx{l}_{qi}_{hp}_{p}")
                            for p in range(2)
                        ]
                        for kb in range(KB):
                            half, tb = kb // TPH, kb % TPH
                            sc = psS.tile(
                                [128, 2 * qn], F32, tag="sc", name=f"sc{l}_{qi}_{hp}_{kb}"
                            )
                            for par_i in range(2):
                                b0 = 64 * par_i
                                nc.tensor.matmul(
                                    sc[:, par_i * qn : par_i * qn + qn],
                                    kT[half][hp][b0 : b0 + 64, tb * 128 : (tb + 1) * 128],
                                    qT[hp][b0 : b0 + 64, q0 : q0 + qn],
                                    start=True,
                                    stop=True,
                                    tile_position=(b0, 0),
                                )
                            ex = expp.tile(
                                [128, 2 * qn], BF16, tag="exp", name=f"ex{l}_{qi}_{hp}_{kb}"
                            )
                            nc.scalar.activation(
                                ex[:],
                                sc[:],
                                AF.Exp,
                                bias=mask_sb[:, kb : kb + 1],
                                scale=float(1.0 / np.sqrt(DH)),
                            )
                            for par_i in range(2):
                                head = 2 * hp + par_i
                                nc.tensor.matmul(
                                    ctx[par_i][:, :],
                                    v_sb[kb][:, head * (DH + 1) : (head + 1) * (DH + 1)],
                                    ex[:, par_i * qn : par_i * qn + qn],
                                    start=(kb == 0),
                                    stop=(kb == KB - 1),
                                )
                        # normalize: ctx[0:64] * (1/sumexp) broadcast over partitions
                        for par_i in range(2):
                            rec_b = rowb.tile(
                                [1, qn], BF16, tag="recb", name=f"rb{l}_{qi}_{hp}_{par_i}"
                            )
                            with nc.allow_low_precision("softmax denom in bf16"):
                                nc.vector.reciprocal(rec_b[:], ctx[par_i][64:65, :])
                            bpool2, btag2 = (psW, "w512") if qi == 0 else (psS, "sc")
                            bc = bpool2.tile(
                                [64, qn], F32, tag=btag2, name=f"bc{l}_{qi}_{hp}_{par_i}"
                            )
                            nc.tensor.matmul(
                                bc[:],
                                ones_r[0:1, 0:64],
                                rec_b[0:1, :],
                                start=True,
                                stop=True,
                            )
                            bcs = expp.tile(
                                [64, qn], BF16, tag="bcs", bufs=2, name=f"bcs{l}_{qi}_{hp}_{par_i}"
                            )
                            nc.vector.tensor_copy(bcs[:], bc[:])
                            b0 = 64 * par_i
                            nc.vector.tensor_tensor(
                                ctxT[hp][b0 : b0 + 64, q0 : q0 + qn],
                                ctx[par_i][0:64, :],
                                bcs[:],
                                op=AluOpType.mult,
                            )

                    # ---- O projection + residual -> lnin ----
                    _ph(f'O_proj{qi}', l, nc)
                    wo_t = [
                        wp.tile([128, Hd], BF16, tag="w", name=f"wo{l}_{qi}_{ic}")
                        for ic in range(HC)
                    ]
                    for ic in range(HC):
                        nc.sync.dma_start(
                            wo_t[ic][:], wo_d[l, ic * 128 : (ic + 1) * 128, :]
                        )
                    for hc in range(HC):
                        wpool, wtag = (psS, "sc") if qi == 1 and hc % 2 else (psW, "w512")
                        ps = wpool.tile([128, qn], F32, tag=wtag, name=f"pso{l}_{qi}_{hc}")
                        for ic in range(HC):
                            nc.tensor.matmul(
                                ps[:],
                                wo_t[ic][:, hc * 128 : (hc + 1) * 128],
                                ctxT[ic][:, q0 : q0 + qn],
                                start=(ic == 0),
                                stop=(ic == HC - 1),
                            )
                        nc.vector.scalar_tensor_tensor(
                            lnin[hc][:, q0 : q0 + qn],
                            ps[:],
                            par_t[:, O_BO + hc : O_BO + hc + 1],
                            h_m[hc][:, q0 : q0 + qn],
                            op0=AluOpType.add,
                            op1=AluOpType.add,
                        )

                    _ph(f'LN1_{qi}', l, nc)
                    layer_norm(q0, qn, O_L1W, O_L1B, "a")

                    # ---- FFN for this q-half ----
                    _ph(f'FFN{qi}', l, nc)
                    for oc in range(FC):
                        wi_t = wip.tile([128, Hd], BF16, tag="wi", name=f"wi{l}_{qi}_{oc}")
                        nc.sync.dma_start(wi_t[:], wi_d[l, oc])
                        # the trailing half's FFN runs after attention is done,
                        # so the idle score banks double as extra accumulators
                        wpool, wtag = (psS, "sc") if qi == 1 and oc % 2 else (psW, "w512")
                        ps = wpool.tile([128, qn], F32, tag=wtag, name=f"psf{l}_{qi}_{oc}")
                        for ic in range(HC):
                            nc.tensor.matmul(
                                ps[:],
                                wi_t[:, ic * 128 : (ic + 1) * 128],
                                h_bf[ic][:, q0 : q0 + qn],
                                start=(ic == 0),
                                stop=(ic == HC - 1),
                            )
                        nc.scalar.activation(
                            ffT(oc, q0)[:, 0:qn], ps[:], AF.Gelu,
                            bias=bi_t[:, oc : oc + 1],
                        )
                    for hc in range(HC):
                        FH = FF // 2
                        wf_t = [
                            wfp.tile([128, FH], BF16, tag="wf", name=f"wf{l}_{qi}_{hc}_{h}")
                            for h in range(2)
                        ]
                        for h in range(2):
                            nc.sync.dma_start(wf_t[h][:], wf_d[l, hc][:, h * FH : (h + 1) * FH])
                        wpool, wtag = (psS, "sc") if qi == 1 and hc % 2 else (psW, "w512")
                        ps = wpool.tile([128, qn], F32, tag=wtag, name=f"psg{l}_{qi}_{hc}")
                        for fc in range(FC):
                            h, fo = fc // (FC // 2), fc % (FC // 2)
                            nc.tensor.matmul(
                                ps[:],
                                wf_t[h][:, fo * 128 : (fo + 1) * 128],
                                ffT(fc, q0)[:, 0:qn],
                                start=(fc == 0),
                                stop=(fc == FC - 1),
                            )
                        nc.vector.scalar_tensor_tensor(
                            lnin[hc][:, q0 : q0 + qn],
                            ps[:],
                            par_t[:, O_BF + hc : O_BF + hc + 1],
                            h_m[hc][:, q0 : q0 + qn],
                            op0=AluOpType.add,
                            op1=AluOpType.add,
                        )

                    _ph(f'LN2_{qi}', l, nc)
                    layer_norm(q0, qn, O_L2W, O_L2B, "b")

            _ph('output', 99, nc)
            # ------------- output (transpose back to token-major) -------------
            for tb in range(TB):
                ysb = scr.tile([128, Hd], F32, tag="scr", name=f"ysb{tb}")
                for hc in range(HC):
                    tpool, ttag = (psS, "sc") if hc % 2 else (psW, "w512")
                    pst = tpool.tile([128, 128], F32, tag=ttag, name=f"yp{tb}_{hc}")
                    nc.tensor.transpose(
                        pst[:], h_m[hc][:, tb * 128 : (tb + 1) * 128], ident[:]
                    )
                    nc.vector.tensor_copy(ysb[:, hc * 128 : (hc + 1) * 128], pst[:])
                nc.sync.dma_start(y_d[tb * 128 : (tb + 1) * 128, :], ysb[:])

        for fr in reversed(_frees):
            fr()

    nc.compile()
    return nc


# ---------------------------------------------------------------------------
# host-side prep + execution
# ---------------------------------------------------------------------------


def prep_shared_inputs(cfg: Cfg, d: dict) -> dict:
    """Inputs identical on every core (weights)."""
    L, Hd, FF, HC, FC = cfg.L, cfg.Hd, cfg.FF, cfg.HC, cfg.FC

    def colpack(x, n):  # [L, n*128] -> [L, 128, n]
        return np.ascontiguousarray(
            np.asarray(x, np.float32).reshape(L, n, 128).transpose(0, 2, 1)
        )

    par = np.concatenate(
        [
            colpack(d["bq"], HC),
            colpack(d["bk"], HC),
            colpack(d["bo"], HC),
            colpack(d["bf"], HC),
            colpack(d["ln1_w"], HC),
            colpack(d["ln1_b"], HC),
            colpack(d["ln2_w"], HC),
            colpack(d["ln2_b"], HC),
        ],
        axis=2,
    )
    # wi[l, oc, p, ic*128+j] = Wi[l, ic*128+p, oc*128+j] (contiguous DMA lines)
    wi_r = np.ascontiguousarray(
        np.asarray(d["Wi"], np.float32)
        .reshape(L, HC, 128, FC, 128)
        .transpose(0, 3, 2, 1, 4)
        .reshape(L, FC, 128, Hd)
        .astype(BF)
    )
    # wf[l, oc2, p, fc*128+j] = Wf[l, fc*128+p, oc2*128+j]
    wf_r = np.ascontiguousarray(
        np.asarray(d["Wf"], np.float32)
        .reshape(L, FC, 128, HC, 128)
        .transpose(0, 3, 2, 1, 4)
        .reshape(L, HC, 128, FF)
        .astype(BF)
    )
    return {
        "wq": np.asarray(d["Wq"], np.float32).astype(BF),
        "wk": np.asarray(d["Wk"], np.float32).astype(BF),
        "wv": np.asarray(d["Wv"], np.float32).astype(BF),
        "wo": np.asarray(d["Wo"], np.float32).astype(BF),
        "wi": wi_r,
        "wf": wf_r,
        "par": par,
        "bi": colpack(d["bi"], FC),
        "bv": np.asarray(d["bv"], np.float32).astype(BF)[:, None, :],
        "wemb": np.asarray(d["word_emb"], np.float32).astype(BF),
        "lne": np.concatenate(
            [
                np.asarray(d["ln_e_w"], np.float32).reshape(HC, 128).T,
                np.asarray(d["ln_e_b"], np.float32).reshape(HC, 128).T,
            ],
            axis=1,
        ),
    }


def prep_core_inputs(cfg: Cfg, core: int, d: dict, shared: dict) -> dict:
    TOK, TB, KB = cfg.TOK, cfg.TB, cfg.KB
    b, hh = core // cfg.NHALF, core % cfg.NHALF
    ids = np.asarray(d["input_ids"], np.int32)[b, hh * TOK : (hh + 1) * TOK]
    mask = np.asarray(d["attention_mask"], np.float32)[b, 0, 0, :]
    pos = (
        np.asarray(d["pos_emb"], np.float32)[hh * TOK : (hh + 1) * TOK]
        + np.asarray(d["type_emb"], np.float32)[0][None, :]
    )
    m = dict(shared)
    m["pos"] = np.ascontiguousarray(pos, dtype=np.float32)
    m["ids"] = np.ascontiguousarray(ids.reshape(TB, 128).T)
    m["mask"] = np.ascontiguousarray(mask.reshape(KB, 128).T)
    return m


_CACHE: dict = {}


def kernel(**inputs) -> np.ndarray:
    cfg = Cfg()
    B = inputs["input_ids"].shape[0]
    if "nc" not in _CACHE:
        _CACHE["nc"] = build(cfg)
    nc = _CACHE["nc"]
    shared = prep_shared_inputs(cfg, inputs)
    in_maps = [prep_core_inputs(cfg, c, inputs, shared) for c in range(cfg.n_cores)]
    res = run_bass_kernel_spmd(nc, in_maps, core_ids=list(range(cfg.n_cores)))
    out = np.zeros((B, cfg.S, cfg.Hd), np.float32)
    for c in range(cfg.n_cores):
        b, hh = c // cfg.NHALF, c % cfg.NHALF
        out[b, hh * cfg.TOK : (hh + 1) * cfg.TOK, :] = res.results[c]["y"]
    return out



# revision 11
# speedup vs baseline: 1.4837x; 1.4837x over previous
"""BERT-base encoder (12L, B=4, S=2048, H=768) on 8 Trainium2 NeuronCores.

Sharding: 8 shards of 1024 tokens each (sample b = core//2, seq-half = core%2).
Per layer, each core computes K/V for its own 1024 tokens and AllGathers them
within the core pair owning the sample, so attention sees the full 2048-token
context while all other work stays perfectly data-parallel.

On-chip layout: activations are kept feature-major (h^T, [768, 1024]) so every
projection is a plain accumulated matmul with the stored [in, out] weights as
the stationary operand. Attention computes scores^T ([k, q]) so the
probs @ V matmul needs no transposes; softmax normalization uses a ones-column
appended to V (row 64 of the ctx PSUM accumulates sum(exp)) plus a K=1
ones-matmul to broadcast 1/sumexp across partitions. LayerNorm statistics are
computed with ones-vector matmuls (sums over the partition dim land in PSUM).

Everything after the K/V AllGather is processed in two independent 512-token
q-halves (attention -> O -> LN1 -> FFN -> LN2 per half), which lets the Tile
scheduler overlap the scalar-engine-bound softmax exp of one half with the
PE-bound FFN of the other, and the next layer's K/V projections with the
trailing half's FFN.
"""

import dataclasses

import numpy as np
import ml_dtypes

import concourse.bass as bass
import concourse.tile as tile
from concourse import bacc, mybir
from concourse.bass import IndirectOffsetOnAxis
from concourse.bass_utils import run_bass_kernel_spmd
from concourse.masks import make_identity
from concourse.alu_op_type import AluOpType

F32 = mybir.dt.float32
BF16 = mybir.dt.bfloat16
I32 = mybir.dt.int32
AF = mybir.ActivationFunctionType
BF = ml_dtypes.bfloat16

PHASE_HOOK = None  # optional (tag, layer, nc) callback for profiling builds


def _ph(tag, l, nc):
    if PHASE_HOOK is not None:
        PHASE_HOOK(tag, l, nc)


@dataclasses.dataclass
class Cfg:
    L: int = 12
    NH: int = 12
    DH: int = 64
    FF: int = 3072
    V: int = 30522
    TOK: int = 1024  # tokens per core
    S: int = 2048  # full sequence
    n_cores: int = 8
    EPS: float = 1e-12

    @property
    def Hd(self):
        return self.NH * self.DH

    @property
    def HC(self):
        return self.Hd // 128  # hidden chunks == head pairs

    @property
    def TB(self):
        return self.TOK // 128

    @property
    def KB(self):
        return self.S // 128

    @property
    def FC(self):
        return self.FF // 128

    @property
    def NHALF(self):
        return self.S // self.TOK  # cores per sample

    @property
    def QS(self):
        return [(q, min(512, self.TOK - q)) for q in range(0, self.TOK, 512)]


def _ns(total, size):
    return [(s, min(size, total - s)) for s in range(0, total, size)]


def build(cfg: Cfg, fake_cc: bool = False):
    L, NH, DH, FF, V = cfg.L, cfg.NH, cfg.DH, cfg.FF, cfg.V
    TOK, S, Hd = cfg.TOK, cfg.S, cfg.Hd
    HC, TB, KB, FC, NHALF = cfg.HC, cfg.TB, cfg.KB, cfg.FC, cfg.NHALF
    QS = cfg.QS
    TPH = TOK // 128  # k blocks per gathered half

    nc = bacc.Bacc(
        "TRN2",
        target_bir_lowering=False,
        debug=False,
        enable_asserts=True,
        num_devices=cfg.n_cores,
    )

    # ---------------- DRAM I/O ----------------
    wq_d = nc.dram_tensor("wq", [L, Hd, Hd], BF16, kind="ExternalInput").ap()
    wk_d = nc.dram_tensor("wk", [L, Hd, Hd], BF16, kind="ExternalInput").ap()
    wv_d = nc.dram_tensor("wv", [L, Hd, Hd], BF16, kind="ExternalInput").ap()
    wo_d = nc.dram_tensor("wo", [L, Hd, Hd], BF16, kind="ExternalInput").ap()
    # wi batched 4 output-chunks per DMA row-block: [L, FC//4, 128, 4*Hd]
    wi_d = nc.dram_tensor("wi", [L, FC // 4, 128, 4 * Hd], BF16, kind="ExternalInput").ap()
    wf_d = nc.dram_tensor("wf", [L, HC, 128, FF], BF16, kind="ExternalInput").ap()
    # packed per-layer params: bq|bk|bo|bf|l1w|l1b|l2w|l2b, each HC cols
    par_d = nc.dram_tensor("par", [L, 128, 8 * HC], F32, kind="ExternalInput").ap()
    bi_d = nc.dram_tensor("bi", [L, 128, FC], F32, kind="ExternalInput").ap()
    bv_d = nc.dram_tensor("bv", [L, 1, Hd], BF16, kind="ExternalInput").ap()
    we_d = nc.dram_tensor("wemb", [V, Hd], BF16, kind="ExternalInput").ap()
    pos_d = nc.dram_tensor("pos", [TOK, Hd], F32, kind="ExternalInput").ap()
    # embedding LN gain|bias packed column-wise: [128, w0..w5 b0..b5]
    lne_d = nc.dram_tensor("lne", [128, 2 * HC], F32, kind="ExternalInput").ap()
    ids_d = nc.dram_tensor("ids", [128, TB], I32, kind="ExternalInput").ap()
    mask_d = nc.dram_tensor("mask", [128, KB], F32, kind="ExternalInput").ap()
    y_d = nc.dram_tensor("y", [TOK, Hd], F32, kind="ExternalOutput").ap()

    VW = NH * (DH + 1)  # V row width incl. interleaved ones columns
    KVN = Hd * TOK + TOK * VW
    kv_in = nc.dram_tensor("kv_in", [KVN], BF16, kind="Internal").ap()
    kv_out = nc.dram_tensor("kv_out", [NHALF, KVN], BF16, kind="Internal").ap()
    kvi_K = kv_in[0 : Hd * TOK].rearrange("(h t) -> h t", t=TOK)
    kvi_V = kv_in[Hd * TOK :].rearrange("(t w) -> t w", w=VW)

    groups = [
        [g * NHALF + i for i in range(NHALF)] for g in range(cfg.n_cores // NHALF)
    ]

    with tile.TileContext(nc) as tc:
        # ---------------- persistent SBUF ----------------
        _frees = []  # keep pool-release closures alive for the whole build

        def single(name, shape, dtype):
            t, fr = tc.tile(shape, dtype, name=name)
            _frees.append(fr)
            return t

        h_m = [single(f"h_m{i}", [128, TOK], F32) for i in range(HC)]
        # pre-LN residual stream kept in bf16: it is only ever read for LN
        # statistics and the (x - mean) recentering, so bf16 precision is
        # enough and the DVE gets its 2x mode on every read
        lnin = [single(f"lnin{i}", [128, TOK], BF16) for i in range(HC)]
        h_bf = [single(f"h_bf{i}", [128, TOK], BF16) for i in range(HC)]
        qT = [single(f"qT{i}", [128, TOK], BF16) for i in range(HC)]
        kT = [
            [single(f"kT{h}_{i}", [128, TOK], BF16) for i in range(HC)]
            for h in range(NHALF)
        ]
        v_sb = [single(f"v_sb{i}", [128, NH * (DH + 1)], BF16) for i in range(KB)]
        ctxT = [single(f"ctxT{i}", [128, TOK], BF16) for i in range(HC)]
        # half of the FFN activation slots; the rest alias qT/ctxT columns of
        # the half currently in its FFN phase (dead there, live for the other)
        ffx = [single(f"ffx{i}", [128, 512], BF16) for i in range(FC - 2 * HC)]

        ids_sb = single("ids_sb", [128, TB], I32)
        mask_sb = single("mask_sb", [128, KB], F32)
        ones_c = single("ones_c", [128, 1], BF16)  # stats lhsT
        ones_r = single("ones_r", [1, 128], BF16)  # broadcast lhsT
        ident = single("ident", [128, 128], F32)
        eps_sb = single("eps_sb", [128, 1], F32)
        nc.vector.memset(eps_sb[:], 1e-12)

        nc.vector.memset(ones_c[:], 1.0)
        nc.vector.memset(ones_r[:], 1.0)
        make_identity(nc, ident[:])
        nc.sync.dma_start(ids_sb[:], ids_d[:, :])
        nc.sync.dma_start(mask_sb[:], mask_d[:, :])

        def ffT(oc, q0):
            """[128, 512] bf16 slot for FFN activation block oc of the q-half
            starting at column q0."""
            if oc < len(ffx):
                return ffx[oc][:, 0:512]
            oc -= len(ffx)
            if oc < HC:
                return qT[oc][:, q0 : q0 + 512]
            return ctxT[oc - HC][:, q0 : q0 + 512]

        with (
            tc.tile_pool(name="wp", bufs=12) as wp,
            tc.tile_pool(name="wip", bufs=2) as wip,
            tc.tile_pool(name="wfp", bufs=2) as wfp,
            tc.tile_pool(name="expp", bufs=3) as expp,
            tc.tile_pool(name="scr", bufs=2) as scr,
            tc.tile_pool(name="rowf", bufs=2) as rowf,
            tc.tile_pool(name="rowb", bufs=2) as rowb,
            tc.tile_pool(name="stg", bufs=2) as stg,
            tc.tile_pool(name="parp", bufs=2) as parp,
            tc.tile_pool(name="psS", bufs=2, space="PSUM") as psS,  # [128,1024] 2 banks
            tc.tile_pool(name="psC", bufs=2, space="PSUM") as psC,  # [65,512] ctx accum
            tc.tile_pool(name="psW", bufs=2, space="PSUM") as psW,  # [128,512] work
        ):
            # ---------------- embedding ----------------
            _ph('embed', -1, nc)
            lne_t, fr1 = tc.tile([128, 2 * HC], F32, name="lne_t")
            nc.sync.dma_start(lne_t[:], lne_d[:, :])

            bn_sub = 256 if Hd % 256 == 0 else 128
            nsub = Hd // bn_sub
            for tb in range(TB):
                emb_g = stg.tile([128, Hd], BF16, tag="vst", name=f"embg{tb}")
                nc.gpsimd.indirect_dma_start(
                    out=emb_g[:],
                    out_offset=None,
                    in_=we_d[:, :],
                    in_offset=IndirectOffsetOnAxis(ap=ids_sb[:, tb : tb + 1], axis=0),
                )
                pos_t = scr.tile([128, Hd], F32, tag="scr", name=f"pos{tb}")
                nc.sync.dma_start(pos_t[:], pos_d[tb * 128 : (tb + 1) * 128, :])
                x = scr.tile([128, Hd], F32, tag="scr", name=f"embx{tb}")
                nc.vector.tensor_copy(x[:], emb_g[:])
                nc.vector.tensor_add(x[:], x[:], pos_t[:])
                # LN over the free (feature) dim
                stats = scr.tile([128, nsub, 6], F32, tag="bst", name=f"bst{tb}")
                for sgi in range(nsub):
                    nc.vector.bn_stats(
                        stats[:, sgi, :], x[:, sgi * bn_sub : (sgi + 1) * bn_sub]
                    )
                mv = scr.tile([128, 2], F32, tag="bmv", name=f"bmv{tb}")
                nc.vector.bn_aggr(mv[:], stats[:])
                rstd = scr.tile([128, 1], F32, tag="brs", name=f"brs{tb}")
                nc.scalar.activation(rstd[:], mv[:, 1:2], AF.Sqrt, bias=eps_sb[:])
                nc.vector.reciprocal(rstd[:], rstd[:])
                nc.vector.tensor_scalar(
                    x[:], x[:], mv[:, 0:1], rstd[:], AluOpType.subtract, AluOpType.mult
                )
                # transpose to feature-major, then gain/bias per feature chunk
                for hc in range(HC):
                    tpool, ttag = (psS, "sc") if hc % 2 else (psW, "w512")
                    pst = tpool.tile([128, 128], F32, tag=ttag, name=f"tp{tb}_{hc}")
                    nc.tensor.transpose(
                        pst[:], x[:, hc * 128 : (hc + 1) * 128], ident[:]
                    )
                    nc.vector.tensor_scalar(
                        h_m[hc][:, tb * 128 : (tb + 1) * 128],
                        pst[:],
                        lne_t[:, hc : hc + 1],
                        lne_t[:, HC + hc : HC + hc + 1],
                        AluOpType.mult,
                        AluOpType.add,
                    )
                    nc.vector.tensor_copy(
                        h_bf[hc][:, tb * 128 : (tb + 1) * 128],
                        h_m[hc][:, tb * 128 : (tb + 1) * 128],
                    )
            fr1()

            # ---------------- layers ----------------
            for l in range(L):
                par_t = parp.tile([128, 8 * HC], F32, tag="par", name=f"par{l}")
                nc.sync.dma_start(par_t[:], par_d[l])
                bi_t = parp.tile([128, FC], F32, tag="bi", name=f"bi{l}")
                nc.sync.dma_start(bi_t[:], bi_d[l])
                bv_t = parp.tile([1, Hd], BF16, tag="bv", bufs=1, name=f"bv{l}")
                nc.sync.dma_start(bv_t[:], bv_d[l])
                O_BQ, O_BK, O_BO, O_BF = 0, HC, 2 * HC, 3 * HC
                O_L1W, O_L1B, O_L2W, O_L2B = 4 * HC, 5 * HC, 6 * HC, 7 * HC

                # ---- K projection -> kv staging ----
                _ph('K_proj', l, nc)
                wk_t = [
                    wp.tile([128, Hd], BF16, tag="w", name=f"wk{l}_{ic}")
                    for ic in range(HC)
                ]
                for ic in range(HC):
                    nc.sync.dma_start(
                        wk_t[ic][:], wk_d[l, ic * 128 : (ic + 1) * 128, :]
                    )
                for hc in range(HC):
                    ks = stg.tile([128, TOK], BF16, tag="kst", name=f"ks{l}_{hc}")
                    for q0, qn in QS:
                        ps = psC.tile([128, qn], F32, tag="ctx", name=f"psk{l}_{hc}_{q0}")
                        for ic in range(HC):
                            nc.tensor.matmul(
                                ps[:],
                                wk_t[ic][:, hc * 128 : (hc + 1) * 128],
                                h_bf[ic][:, q0 : q0 + qn],
                                start=(ic == 0),
                                stop=(ic == HC - 1),
                            )
                        nc.vector.tensor_scalar_add(
                            ks[:, q0 : q0 + qn], ps[:], par_t[:, O_BK + hc : O_BK + hc + 1]
                        )
                    nc.sync.dma_start(kvi_K[hc * 128 : (hc + 1) * 128, :], ks[:])

                # ---- V projection (token-major, bias via K=1 ones matmul) ----
                _ph('V_proj', l, nc)
                wv_t = [
                    wp.tile([128, Hd], BF16, tag="w", name=f"wv{l}_{ic}")
                    for ic in range(HC)
                ]
                for ic in range(HC):
                    nc.sync.dma_start(
                        wv_t[ic][:], wv_d[l, ic * 128 : (ic + 1) * 128, :]
                    )
                for tb in range(TB):
                    vs = stg.tile([128, VW], BF16, tag="vst", name=f"vs{l}_{tb}")
                    vsv = vs[:].rearrange("p (n e) -> p n e", e=DH + 1)
                    for n0, nn in _ns(Hd, 512):
                        ps = psC.tile([128, nn], F32, tag="ctx", name=f"psv{l}_{tb}_{n0}")
                        nc.tensor.matmul(
                            ps[:],
                            ones_r[0:1, 0:128],
                            bv_t[0:1, n0 : n0 + nn],
                            start=True,
                            stop=False,
                        )
                        for ic in range(HC):
                            nc.tensor.matmul(
                                ps[:],
                                h_bf[ic][:, tb * 128 : (tb + 1) * 128],
                                wv_t[ic][:, n0 : n0 + nn],
                                start=False,
                                stop=(ic == HC - 1),
                            )
                        h0, hn = n0 // DH, nn // DH  # head range of this chunk
                        nc.vector.tensor_copy(
                            vsv[:, h0 : h0 + hn, 0:DH],
                            ps[:].rearrange("p (n d) -> p n d", d=DH),
                        )
                    nc.vector.memset(vsv[:, :, DH : DH + 1], 1.0)
                    nc.sync.dma_start(kvi_V[tb * 128 : (tb + 1) * 128, :], vs[:])

                # ---- AllGather K/V within the sample's core group ----
                _ph('AllGather', l, nc)
                if fake_cc:
                    for half in range(NHALF):
                        nc.sync.dma_start(kv_out[half], kv_in[:])
                else:
                    nc.gpsimd.collective_compute(
                        "AllGather",
                        AluOpType.bypass,
                        replica_groups=groups,
                        ins=[kv_in[:].opt()],
                        outs=[kv_out[:].opt()],
                    )

                # ---- Q projection (overlaps the collective) ----
                _ph('Q_proj', l, nc)
                wq_t = [
                    wp.tile([128, Hd], BF16, tag="w", name=f"wq{l}_{ic}")
                    for ic in range(HC)
                ]
                for ic in range(HC):
                    nc.sync.dma_start(
                        wq_t[ic][:], wq_d[l, ic * 128 : (ic + 1) * 128, :]
                    )
                for hc in range(HC):
                    for q0, qn in QS:
                        ps = psC.tile([128, qn], F32, tag="ctx", name=f"psq{l}_{hc}_{q0}")
                        for ic in range(HC):
                            nc.tensor.matmul(
                                ps[:],
                                wq_t[ic][:, hc * 128 : (hc + 1) * 128],
                                h_bf[ic][:, q0 : q0 + qn],
                                start=(ic == 0),
                                stop=(ic == HC - 1),
                            )
                        nc.vector.tensor_scalar_add(
                            qT[hc][:, q0 : q0 + qn], ps[:],
                            par_t[:, O_BQ + hc : O_BQ + hc + 1],
                        )

                # ---- unpack gathered K/V ----
                _ph('unpack_KV', l, nc)
                for half in range(NHALF):
                    kvo_K = kv_out[half, 0 : Hd * TOK].rearrange("(h t) -> h t", t=TOK)
                    kvo_V = kv_out[half, Hd * TOK :].rearrange("(t w) -> t w", w=VW)
                    for hc in range(HC):
                        nc.sync.dma_start(
                            kT[half][hc][:], kvo_K[hc * 128 : (hc + 1) * 128, :]
                        )
                    for tb in range(TPH):
                        kb = half * TPH + tb
                        nc.sync.dma_start(
                            v_sb[kb][:], kvo_V[tb * 128 : (tb + 1) * 128, :]
                        )

                # ---- per q-half: attention -> O -> LN1 -> FFN -> LN2 ----
                def layer_norm(q0, qn, w_off, b_off, tag):
                    # squares scratch lives in qT columns of this half (dead
                    # between this half's attention and next-layer Q proj);
                    # lnin is already bf16 so stats read it directly
                    for hc in range(HC):
                        nc.vector.tensor_mul(
                            qT[hc][:, q0 : q0 + qn],
                            lnin[hc][:, q0 : q0 + qn],
                            lnin[hc][:, q0 : q0 + qn],
                        )
                    s_ps = psW.tile([1, qn], F32, tag="w512", name=f"sps{tag}{l}_{q0}")
                    q_ps = psW.tile([1, qn], F32, tag="w512", name=f"qps{tag}{l}_{q0}")
                    for hc in range(HC):
                        nc.tensor.matmul(
                            s_ps[:],
                            ones_c[:],
                            lnin[hc][:, q0 : q0 + qn],
                            start=(hc == 0),
                            stop=(hc == HC - 1),
                        )
                    for hc in range(HC):
                        nc.tensor.matmul(
                            q_ps[:],
                            ones_c[:],
                            qT[hc][:, q0 : q0 + qn],
                            start=(hc == 0),
                            stop=(hc == HC - 1),
                        )
                    t1 = rowf.tile([1, qn], F32, tag="t1", bufs=1, name=f"t1{tag}{l}_{q0}")
                    nc.scalar.square(t1[:], s_ps[:])
                    nc.vector.scalar_tensor_tensor(
                        t1[:],
                        t1[:],
                        -1.0 / Hd,
                        q_ps[:],
                        op0=AluOpType.mult,
                        op1=AluOpType.add,
                    )
                    # t1 = sumsq - sum^2/Hd ; rstd = 1/sqrt(t1/Hd + eps)
                    nc.scalar.activation(
                        t1[:], t1[:], AF.Sqrt, scale=1.0 / Hd, bias=eps_sb[0:1, :]
                    )
                    nc.vector.reciprocal(t1[:], t1[:])
                    mr_b = rowb.tile(
                        [1, 2 * qn], BF16, tag="mrb", bufs=1, name=f"mr{tag}{l}_{q0}"
                    )
                    nc.vector.tensor_scalar_mul(mr_b[:, 0:qn], s_ps[:], 1.0 / Hd)
                    nc.vector.tensor_copy(mr_b[:, qn : 2 * qn], t1[:])
                    bpool, btag = (psS, "sc") if q0 >= 512 else (psW, "w512")
                    m_bc = bpool.tile([128, qn], F32, tag=btag, name=f"mbc{tag}{l}_{q0}")
                    r_bc = psW.tile([128, qn], F32, tag="w512", name=f"rbc{tag}{l}_{q0}")
                    nc.tensor.matmul(
                        m_bc[:], ones_r[0:1, 0:128], mr_b[0:1, 0:qn],
                        start=True, stop=True,
                    )
                    nc.tensor.matmul(
                        r_bc[:], ones_r[0:1, 0:128], mr_b[0:1, qn : 2 * qn],
                        start=True, stop=True,
                    )
                    for hc in range(HC):
                        t = scr.tile([128, qn], F32, tag="scr", name=f"sc{tag}{l}_{q0}_{hc}")
                        nc.vector.tensor_sub(t[:], lnin[hc][:, q0 : q0 + qn], m_bc[:])
                        nc.vector.tensor_mul(t[:], t[:], r_bc[:])
                        nc.vector.tensor_scalar(
                            h_m[hc][:, q0 : q0 + qn],
                            t[:],
                            par_t[:, w_off + hc : w_off + hc + 1],
                            par_t[:, b_off + hc : b_off + hc + 1],
                            AluOpType.mult,
                            AluOpType.add,
                        )
                        nc.vector.tensor_copy(
                            h_bf[hc][:, q0 : q0 + qn], h_m[hc][:, q0 : q0 + qn]
                        )

                # O weights loaded once per layer, shared by both q-halves
                wo_t = [
                    wp.tile([128, Hd], BF16, tag="w", name=f"wo{l}_{ic}")
                    for ic in range(HC)
                ]
                for ic in range(HC):
                    nc.sync.dma_start(
                        wo_t[ic][:], wo_d[l, ic * 128 : (ic + 1) * 128, :]
                    )

                for qi, (q0, qn) in enumerate(QS):
                    # ---- attention for this q-half ----
                    _ph(f'attn{qi}', l, nc)
                    for hp in range(HC):
                        ctx = [
                            psC.tile([65, qn], F32, tag="ctx", name=f"ctx{l}_{qi}_{hp}_{p}")
                            for p in range(2)
                        ]
                        for kb in range(KB):
                            half, tb = kb // TPH, kb % TPH
                            sc = psS.tile(
                                [128, 2 * qn], F32, tag="sc", name=f"sc{l}_{qi}_{hp}_{kb}"
                            )
                            for par_i in range(2):
                                b0 = 64 * par_i
                                nc.tensor.matmul(
                                    sc[:, par_i * qn : par_i * qn + qn],
                                    kT[half][hp][b0 : b0 + 64, tb * 128 : (tb + 1) * 128],
                                    qT[hp][b0 : b0 + 64, q0 : q0 + qn],
                                    start=True,
                                    stop=True,
                                    tile_position=(b0, 0),
                                )
                            ex = expp.tile(
                                [128, 2 * qn], BF16, tag="exp", name=f"ex{l}_{qi}_{hp}_{kb}"
                            )
                            nc.scalar.activation(
                                ex[:],
                                sc[:],
                                AF.Exp,
                                bias=mask_sb[:, kb : kb + 1],
                                scale=float(1.0 / np.sqrt(DH)),
                            )
                            for par_i in range(2):
                                head = 2 * hp + par_i
                                nc.tensor.matmul(
                                    ctx[par_i][:, :],
                                    v_sb[kb][:, head * (DH + 1) : (head + 1) * (DH + 1)],
                                    ex[:, par_i * qn : par_i * qn + qn],
                                    start=(kb == 0),
                                    stop=(kb == KB - 1),
                                )
                        # normalize: ctx[0:64] * (1/sumexp) broadcast over partitions
                        for par_i in range(2):
                            rec_b = rowb.tile(
                                [1, qn], BF16, tag="recb", name=f"rb{l}_{qi}_{hp}_{par_i}"
                            )
                            with nc.allow_low_precision("softmax denom in bf16"):
                                nc.vector.reciprocal(rec_b[:], ctx[par_i][64:65, :])
                            bpool2, btag2 = (psW, "w512") if qi == 0 else (psS, "sc")
                            bc = bpool2.tile(
                                [64, qn], F32, tag=btag2, name=f"bc{l}_{qi}_{hp}_{par_i}"
                            )
                            nc.tensor.matmul(
                                bc[:],
                                ones_r[0:1, 0:64],
                                rec_b[0:1, :],
                                start=True,
                                stop=True,
                            )
                            bcs = expp.tile(
                                [64, qn], BF16, tag="bcs", bufs=2, name=f"bcs{l}_{qi}_{hp}_{par_i}"
                            )
                            nc.vector.tensor_copy(bcs[:], bc[:])
                            b0 = 64 * par_i
                            nc.vector.tensor_tensor(
                                ctxT[hp][b0 : b0 + 64, q0 : q0 + qn],
                                ctx[par_i][0:64, :],
                                bcs[:],
                                op=AluOpType.mult,
                            )

                    # ---- O projection + residual -> lnin ----
                    _ph(f'O_proj{qi}', l, nc)
                    for hc in range(HC):
                        wpool, wtag = (psS, "sc") if qi == 1 and hc % 2 else (psW, "w512")
                        ps = wpool.tile([128, qn], F32, tag=wtag, name=f"pso{l}_{qi}_{hc}")
                        for ic in range(HC):
                            nc.tensor.matmul(
                                ps[:],
                                wo_t[ic][:, hc * 128 : (hc + 1) * 128],
                                ctxT[ic][:, q0 : q0 + qn],
                                start=(ic == 0),
                                stop=(ic == HC - 1),
                            )
                        nc.vector.scalar_tensor_tensor(
                            lnin[hc][:, q0 : q0 + qn],
                            ps[:],
                            par_t[:, O_BO + hc : O_BO + hc + 1],
                            h_m[hc][:, q0 : q0 + qn],
                            op0=AluOpType.add,
                            op1=AluOpType.add,
                        )

                    _ph(f'LN1_{qi}', l, nc)
                    layer_norm(q0, qn, O_L1W, O_L1B, "a")

                    # ---- FFN for this q-half ----
                    _ph(f'FFN{qi}', l, nc)
                    for oc in range(FC):
                        if oc % 4 == 0:
                            wi_t = wip.tile(
                                [128, 4 * Hd], BF16, tag="wi", name=f"wi{l}_{qi}_{oc}"
                            )
                            nc.sync.dma_start(wi_t[:], wi_d[l, oc // 4])
                        ob = (oc % 4) * Hd
                        # the trailing half's FFN runs after attention is done,
                        # so the idle score banks double as extra accumulators
                        wpool, wtag = (psS, "sc") if qi == 1 and oc % 2 else (psW, "w512")
                        ps = wpool.tile([128, qn], F32, tag=wtag, name=f"psf{l}_{qi}_{oc}")
                        for ic in range(HC):
                            nc.tensor.matmul(
                                ps[:],
                                wi_t[:, ob + ic * 128 : ob + (ic + 1) * 128],
                                h_bf[ic][:, q0 : q0 + qn],
                                start=(ic == 0),
                                stop=(ic == HC - 1),
                            )
                        nc.scalar.activation(
                            ffT(oc, q0)[:, 0:qn], ps[:], AF.Gelu,
                            bias=bi_t[:, oc : oc + 1],
                        )
                    for hc in range(HC):
                        wf_t = wfp.tile([128, FF], BF16, tag="wf", name=f"wf{l}_{qi}_{hc}")
                        nc.sync.dma_start(wf_t[:], wf_d[l, hc])
                        wpool, wtag = (psS, "sc") if qi == 1 and hc % 2 else (psW, "w512")
                        ps = wpool.tile([128, qn], F32, tag=wtag, name=f"psg{l}_{qi}_{hc}")
                        for fc in range(FC):
                            nc.tensor.matmul(
                                ps[:],
                                wf_t[:, fc * 128 : (fc + 1) * 128],
                                ffT(fc, q0)[:, 0:qn],
                                start=(fc == 0),
                                stop=(fc == FC - 1),
                            )
                        nc.vector.scalar_tensor_tensor(
                            lnin[hc][:, q0 : q0 + qn],
                            ps[:],
                            par_t[:, O_BF + hc : O_BF + hc + 1],
                            h_m[hc][:, q0 : q0 + qn],
                            op0=AluOpType.add,
                            op1=AluOpType.add,
                        )

                    _ph(f'LN2_{qi}', l, nc)
                    layer_norm(q0, qn, O_L2W, O_L2B, "b")

            _ph('output', 99, nc)
            # ------------- output (transpose back to token-major) -------------
            for tb in range(TB):
                ysb = scr.tile([128, Hd], F32, tag="scr", name=f"ysb{tb}")
                for hc in range(HC):
                    tpool, ttag = (psS, "sc") if hc % 2 else (psW, "w512")
                    pst = tpool.tile([128, 128], F32, tag=ttag, name=f"yp{tb}_{hc}")
                    nc.tensor.transpose(
                        pst[:], h_m[hc][:, tb * 128 : (tb + 1) * 128], ident[:]
                    )
                    nc.vector.tensor_copy(ysb[:, hc * 128 : (hc + 1) * 128], pst[:])
                nc.sync.dma_start(y_d[tb * 128 : (tb + 1) * 128, :], ysb[:])

        for fr in reversed(_frees):
            fr()

    nc.compile()
    return nc


# ---------------------------------------------------------------------------
# host-side prep + execution
# ---------------------------------------------------------------------------


def prep_shared_inputs(cfg: Cfg, d: dict) -> dict:
    """Inputs identical on every core (weights)."""
    L, Hd, FF, HC, FC = cfg.L, cfg.Hd, cfg.FF, cfg.HC, cfg.FC

    def colpack(x, n):  # [L, n*128] -> [L, 128, n]
        return np.ascontiguousarray(
            np.asarray(x, np.float32).reshape(L, n, 128).transpose(0, 2, 1)
        )

    par = np.concatenate(
        [
            colpack(d["bq"], HC),
            colpack(d["bk"], HC),
            colpack(d["bo"], HC),
            colpack(d["bf"], HC),
            colpack(d["ln1_w"], HC),
            colpack(d["ln1_b"], HC),
            colpack(d["ln2_w"], HC),
            colpack(d["ln2_b"], HC),
        ],
        axis=2,
    )
    # wi[l, oc, p, ic*128+j] = Wi[l, ic*128+p, oc*128+j], then 4 oc per DMA
    # row-block: wi4[l, g, p, m*Hd + ic*128+j] = wi[l, 4g+m, p, ic*128+j]
    wi_r = np.ascontiguousarray(
        np.asarray(d["Wi"], np.float32)
        .reshape(L, HC, 128, FC, 128)
        .transpose(0, 3, 2, 1, 4)
        .reshape(L, FC // 4, 4, 128, Hd)
        .transpose(0, 1, 3, 2, 4)
        .reshape(L, FC // 4, 128, 4 * Hd)
        .astype(BF)
    )
    # wf[l, oc2, p, fc*128+j] = Wf[l, fc*128+p, oc2*128+j]
    wf_r = np.ascontiguousarray(
        np.asarray(d["Wf"], np.float32)
        .reshape(L, FC, 128, HC, 128)
        .transpose(0, 3, 2, 1, 4)
        .reshape(L, HC, 128, FF)
        .astype(BF)
    )
    return {
        "wq": np.asarray(d["Wq"], np.float32).astype(BF),
        "wk": np.asarray(d["Wk"], np.float32).astype(BF),
        "wv": np.asarray(d["Wv"], np.float32).astype(BF),
        "wo": np.asarray(d["Wo"], np.float32).astype(BF),
        "wi": wi_r,
        "wf": wf_r,
        "par": par,
        "bi": colpack(d["bi"], FC),
        "bv": np.asarray(d["bv"], np.float32).astype(BF)[:, None, :],
        "wemb": np.asarray(d["word_emb"], np.float32).astype(BF),
        "lne": np.concatenate(
            [
                np.asarray(d["ln_e_w"], np.float32).reshape(HC, 128).T,
                np.asarray(d["ln_e_b"], np.float32).reshape(HC, 128).T,
            ],
            axis=1,
        ),
    }


def prep_core_inputs(cfg: Cfg, core: int, d: dict, shared: dict) -> dict:
    TOK, TB, KB = cfg.TOK, cfg.TB, cfg.KB
    b, hh = core // cfg.NHALF, core % cfg.NHALF
    ids = np.asarray(d["input_ids"], np.int32)[b, hh * TOK : (hh + 1) * TOK]
    mask = np.asarray(d["attention_mask"], np.float32)[b, 0, 0, :]
    pos = (
        np.asarray(d["pos_emb"], np.float32)[hh * TOK : (hh + 1) * TOK]
        + np.asarray(d["type_emb"], np.float32)[0][None, :]
    )
    m = dict(shared)
    m["pos"] = np.ascontiguousarray(pos, dtype=np.float32)
    m["ids"] = np.ascontiguousarray(ids.reshape(TB, 128).T)
    m["mask"] = np.ascontiguousarray(mask.reshape(KB, 128).T)
    return m


_CACHE: dict = {}


def kernel(**inputs) -> np.ndarray:
    cfg = Cfg()
    B = inputs["input_ids"].shape[0]
    if "nc" not in _CACHE:
        _CACHE["nc"] = build(cfg)
    nc = _CACHE["nc"]
    shared = prep_shared_inputs(cfg, inputs)
    in_maps = [prep_core_inputs(cfg, c, inputs, shared) for c in range(cfg.n_cores)]
    res = run_bass_kernel_spmd(nc, in_maps, core_ids=list(range(cfg.n_cores)))
    out = np.zeros((B, cfg.S, cfg.Hd), np.float32)
    for c in range(cfg.n_cores):
        b, hh = c // cfg.NHALF, c % cfg.NHALF
        out[b, hh * cfg.TOK : (hh + 1) * cfg.TOK, :] = res.results[c]["y"]
    return out



# revision 14
# speedup vs baseline: 1.5196x; 1.0242x over previous
"""BERT-base encoder (12L, B=4, S=2048, H=768) on 8 Trainium2 NeuronCores.

Sharding: 8 shards of 1024 tokens each (sample b = core//2, seq-half = core%2).
Per layer, each core computes K/V for its own 1024 tokens and AllGathers them
within the core pair owning the sample, so attention sees the full 2048-token
context while all other work stays perfectly data-parallel.

On-chip layout: activations are kept feature-major (h^T, [768, 1024]) so every
projection is a plain accumulated matmul with the stored [in, out] weights as
the stationary operand. Attention computes scores^T ([k, q]) so the
probs @ V matmul needs no transposes; softmax normalization uses a ones-column
appended to V (row 64 of the ctx PSUM accumulates sum(exp)) plus a K=1
ones-matmul to broadcast 1/sumexp across partitions. LayerNorm statistics are
computed with ones-vector matmuls (sums over the partition dim land in PSUM).

Everything after the K/V AllGather is processed in two independent 512-token
q-halves (attention -> O -> LN1 -> FFN -> LN2 per half), which lets the Tile
scheduler overlap the scalar-engine-bound softmax exp of one half with the
PE-bound FFN of the other, and the next layer's K/V projections with the
trailing half's FFN.
"""

import dataclasses

import numpy as np
import ml_dtypes

import concourse.bass as bass
import concourse.tile as tile
from concourse import bacc, mybir
from concourse.bass import IndirectOffsetOnAxis
from concourse.bass_utils import run_bass_kernel_spmd
from concourse.masks import make_identity
from concourse.alu_op_type import AluOpType

F32 = mybir.dt.float32
BF16 = mybir.dt.bfloat16
I32 = mybir.dt.int32
AF = mybir.ActivationFunctionType
BF = ml_dtypes.bfloat16

PHASE_HOOK = None  # optional (tag, layer, nc) callback for profiling builds


def _ph(tag, l, nc):
    if PHASE_HOOK is not None:
        PHASE_HOOK(tag, l, nc)


@dataclasses.dataclass
class Cfg:
    L: int = 12
    NH: int = 12
    DH: int = 64
    FF: int = 3072
    V: int = 30522
    TOK: int = 1024  # tokens per core
    S: int = 2048  # full sequence
    n_cores: int = 8
    EPS: float = 1e-12

    @property
    def Hd(self):
        return self.NH * self.DH

    @property
    def HC(self):
        return self.Hd // 128  # hidden chunks == head pairs

    @property
    def TB(self):
        return self.TOK // 128

    @property
    def KB(self):
        return self.S // 128

    @property
    def FC(self):
        return self.FF // 128

    @property
    def NHALF(self):
        return self.S // self.TOK  # cores per sample

    @property
    def QS(self):
        return [(q, min(512, self.TOK - q)) for q in range(0, self.TOK, 512)]


def _ns(total, size):
    return [(s, min(size, total - s)) for s in range(0, total, size)]


def build(cfg: Cfg, fake_cc: bool = False):
    L, NH, DH, FF, V = cfg.L, cfg.NH, cfg.DH, cfg.FF, cfg.V
    TOK, S, Hd = cfg.TOK, cfg.S, cfg.Hd
    HC, TB, KB, FC, NHALF = cfg.HC, cfg.TB, cfg.KB, cfg.FC, cfg.NHALF
    QS = cfg.QS
    TPH = TOK // 128  # k blocks per gathered half

    nc = bacc.Bacc(
        "TRN2",
        target_bir_lowering=False,
        debug=False,
        enable_asserts=True,
        num_devices=cfg.n_cores,
    )

    # ---------------- DRAM I/O ----------------
    wq_d = nc.dram_tensor("wq", [L, Hd, Hd], BF16, kind="ExternalInput").ap()
    wk_d = nc.dram_tensor("wk", [L, Hd, Hd], BF16, kind="ExternalInput").ap()
    wv_d = nc.dram_tensor("wv", [L, Hd, Hd], BF16, kind="ExternalInput").ap()
    wo_d = nc.dram_tensor("wo", [L, Hd, Hd], BF16, kind="ExternalInput").ap()
    # wi batched 2 output-chunks per DMA row-block: [L, FC//2, 128, 2*Hd]
    wi_d = nc.dram_tensor("wi", [L, FC // 2, 128, 2 * Hd], BF16, kind="ExternalInput").ap()
    wf_d = nc.dram_tensor("wf", [L, HC, 128, FF], BF16, kind="ExternalInput").ap()
    # packed per-layer params: bq|bk|bo|bf|l1w|l1b|l2w|l2b, each HC cols
    par_d = nc.dram_tensor("par", [L, 128, 8 * HC], F32, kind="ExternalInput").ap()
    bi_d = nc.dram_tensor("bi", [L, 128, FC], F32, kind="ExternalInput").ap()
    bv_d = nc.dram_tensor("bv", [L, 1, Hd], BF16, kind="ExternalInput").ap()
    we_d = nc.dram_tensor("wemb", [V, Hd], BF16, kind="ExternalInput").ap()
    pos_d = nc.dram_tensor("pos", [TOK, Hd], F32, kind="ExternalInput").ap()
    # embedding LN gain|bias packed column-wise: [128, w0..w5 b0..b5]
    lne_d = nc.dram_tensor("lne", [128, 2 * HC], F32, kind="ExternalInput").ap()
    ids_d = nc.dram_tensor("ids", [128, TB], I32, kind="ExternalInput").ap()
    mask_d = nc.dram_tensor("mask", [128, KB], F32, kind="ExternalInput").ap()
    y_d = nc.dram_tensor("y", [TOK, Hd], F32, kind="ExternalOutput").ap()

    VW = NH * (DH + 1)  # V row width incl. interleaved ones columns
    KVN = Hd * TOK + TOK * VW
    kv_in = nc.dram_tensor("kv_in", [KVN], BF16, kind="Internal").ap()
    kv_out = nc.dram_tensor("kv_out", [NHALF, KVN], BF16, kind="Internal").ap()
    kvi_K = kv_in[0 : Hd * TOK].rearrange("(h t) -> h t", t=TOK)
    kvi_V = kv_in[Hd * TOK :].rearrange("(t w) -> t w", w=VW)

    groups = [
        [g * NHALF + i for i in range(NHALF)] for g in range(cfg.n_cores // NHALF)
    ]

    with tile.TileContext(nc) as tc:
        # ---------------- persistent SBUF ----------------
        _frees = []  # keep pool-release closures alive for the whole build

        def single(name, shape, dtype):
            t, fr = tc.tile(shape, dtype, name=name)
            _frees.append(fr)
            return t

        h_m = [single(f"h_m{i}", [128, TOK], F32) for i in range(HC)]
        lnin = [single(f"lnin{i}", [128, TOK], F32) for i in range(HC)]
        h_bf = [single(f"h_bf{i}", [128, TOK], BF16) for i in range(HC)]
        qT = [single(f"qT{i}", [128, TOK], BF16) for i in range(HC)]
        kT = [
            [single(f"kT{h}_{i}", [128, TOK], BF16) for i in range(HC)]
            for h in range(NHALF)
        ]
        v_sb = [single(f"v_sb{i}", [128, NH * (DH + 1)], BF16) for i in range(KB)]
        ctxT = [single(f"ctxT{i}", [128, TOK], BF16) for i in range(HC)]
        # half of the FFN activation slots; the rest alias qT/ctxT columns of
        # the half currently in its FFN phase (dead there, live for the other)
        ffx = [single(f"ffx{i}", [128, 512], BF16) for i in range(FC - 2 * HC)]

        ids_sb = single("ids_sb", [128, TB], I32)
        mask_sb = single("mask_sb", [128, KB], F32)
        ones_c = single("ones_c", [128, 1], BF16)  # stats lhsT
        ones_r = single("ones_r", [1, 128], BF16)  # broadcast lhsT
        ident = single("ident", [128, 128], F32)
        eps_sb = single("eps_sb", [128, 1], F32)
        nc.vector.memset(eps_sb[:], 1e-12)

        nc.vector.memset(ones_c[:], 1.0)
        nc.vector.memset(ones_r[:], 1.0)
        make_identity(nc, ident[:])
        nc.sync.dma_start(ids_sb[:], ids_d[:, :])
        nc.sync.dma_start(mask_sb[:], mask_d[:, :])

        def ffT(oc, q0):
            """[128, 512] bf16 slot for FFN activation block oc of the q-half
            starting at column q0."""
            if oc < len(ffx):
                return ffx[oc][:, 0:512]
            oc -= len(ffx)
            if oc < HC:
                return qT[oc][:, q0 : q0 + 512]
            return ctxT[oc - HC][:, q0 : q0 + 512]

        with (
            tc.tile_pool(name="wp", bufs=9) as wp,
            tc.tile_pool(name="wip", bufs=3) as wip,
            tc.tile_pool(name="wfp", bufs=2) as wfp,
            tc.tile_pool(name="expp", bufs=3) as expp,
            tc.tile_pool(name="scr", bufs=2) as scr,
            tc.tile_pool(name="rowf", bufs=2) as rowf,
            tc.tile_pool(name="rowb", bufs=2) as rowb,
            tc.tile_pool(name="stg", bufs=2) as stg,
            tc.tile_pool(name="parp", bufs=2) as parp,
            tc.tile_pool(name="psS", bufs=2, space="PSUM") as psS,  # [128,1024] 2 banks
            tc.tile_pool(name="psC", bufs=2, space="PSUM") as psC,  # [65,512] ctx accum
            tc.tile_pool(name="psW", bufs=2, space="PSUM") as psW,  # [128,512] work
        ):
            # ---------------- embedding ----------------
            _ph('embed', -1, nc)
            lne_t, fr1 = tc.tile([128, 2 * HC], F32, name="lne_t")
            nc.sync.dma_start(lne_t[:], lne_d[:, :])

            bn_sub = 256 if Hd % 256 == 0 else 128
            nsub = Hd // bn_sub
            for tb in range(TB):
                emb_g = stg.tile([128, Hd], BF16, tag="vst", name=f"embg{tb}")
                nc.gpsimd.indirect_dma_start(
                    out=emb_g[:],
                    out_offset=None,
                    in_=we_d[:, :],
                    in_offset=IndirectOffsetOnAxis(ap=ids_sb[:, tb : tb + 1], axis=0),
                )
                pos_t = scr.tile([128, Hd], F32, tag="scr", name=f"pos{tb}")
                nc.sync.dma_start(pos_t[:], pos_d[tb * 128 : (tb + 1) * 128, :])
                x = scr.tile([128, Hd], F32, tag="scr", name=f"embx{tb}")
                nc.vector.tensor_copy(x[:], emb_g[:])
                nc.vector.tensor_add(x[:], x[:], pos_t[:])
                # LN over the free (feature) dim
                stats = scr.tile([128, nsub, 6], F32, tag="bst", name=f"bst{tb}")
                for sgi in range(nsub):
                    nc.vector.bn_stats(
                        stats[:, sgi, :], x[:, sgi * bn_sub : (sgi + 1) * bn_sub]
                    )
                mv = scr.tile([128, 2], F32, tag="bmv", name=f"bmv{tb}")
                nc.vector.bn_aggr(mv[:], stats[:])
                rstd = scr.tile([128, 1], F32, tag="brs", name=f"brs{tb}")
                nc.scalar.activation(rstd[:], mv[:, 1:2], AF.Sqrt, bias=eps_sb[:])
                nc.vector.reciprocal(rstd[:], rstd[:])
                nc.vector.tensor_scalar(
                    x[:], x[:], mv[:, 0:1], rstd[:], AluOpType.subtract, AluOpType.mult
                )
                # transpose to feature-major, then gain/bias per feature chunk
                for hc in range(HC):
                    tpool, ttag = (psS, "sc") if hc % 2 else (psW, "w512")
                    pst = tpool.tile([128, 128], F32, tag=ttag, name=f"tp{tb}_{hc}")
                    nc.tensor.transpose(
                        pst[:], x[:, hc * 128 : (hc + 1) * 128], ident[:]
                    )
                    nc.vector.tensor_scalar(
                        h_m[hc][:, tb * 128 : (tb + 1) * 128],
                        pst[:],
                        lne_t[:, hc : hc + 1],
                        lne_t[:, HC + hc : HC + hc + 1],
                        AluOpType.mult,
                        AluOpType.add,
                    )
                    nc.vector.tensor_copy(
                        h_bf[hc][:, tb * 128 : (tb + 1) * 128],
                        h_m[hc][:, tb * 128 : (tb + 1) * 128],
                    )
            fr1()

            # ---------------- layers ----------------
            for l in range(L):
                par_t = parp.tile([128, 8 * HC], F32, tag="par", name=f"par{l}")
                nc.sync.dma_start(par_t[:], par_d[l])
                bi_t = parp.tile([128, FC], F32, tag="bi", name=f"bi{l}")
                nc.sync.dma_start(bi_t[:], bi_d[l])
                bv_t = parp.tile([1, Hd], BF16, tag="bv", bufs=1, name=f"bv{l}")
                nc.sync.dma_start(bv_t[:], bv_d[l])
                O_BQ, O_BK, O_BO, O_BF = 0, HC, 2 * HC, 3 * HC
                O_L1W, O_L1B, O_L2W, O_L2B = 4 * HC, 5 * HC, 6 * HC, 7 * HC

                # ---- K projection -> kv staging ----
                _ph('K_proj', l, nc)
                wk_t = [
                    wp.tile([128, Hd], BF16, tag="w", name=f"wk{l}_{ic}")
                    for ic in range(HC)
                ]
                for ic in range(HC):
                    nc.sync.dma_start(
                        wk_t[ic][:], wk_d[l, ic * 128 : (ic + 1) * 128, :]
                    )
                for hc in range(HC):
                    # stage K through ctxT[hc], dead since last layer's O proj
                    ks = ctxT[hc]
                    for q0, qn in QS:
                        ps = psC.tile([128, qn], F32, tag="ctx", name=f"psk{l}_{hc}_{q0}")
                        for ic in range(HC):
                            nc.tensor.matmul(
                                ps[:],
                                wk_t[ic][:, hc * 128 : (hc + 1) * 128],
                                h_bf[ic][:, q0 : q0 + qn],
                                start=(ic == 0),
                                stop=(ic == HC - 1),
                            )
                        nc.vector.tensor_scalar_add(
                            ks[:, q0 : q0 + qn], ps[:], par_t[:, O_BK + hc : O_BK + hc + 1]
                        )
                    nc.sync.dma_start(kvi_K[hc * 128 : (hc + 1) * 128, :], ks[:])

                # ---- V projection (token-major, bias via K=1 ones matmul) ----
                _ph('V_proj', l, nc)
                wv_t = [
                    wp.tile([128, Hd], BF16, tag="w", name=f"wv{l}_{ic}")
                    for ic in range(HC)
                ]
                for ic in range(HC):
                    nc.sync.dma_start(
                        wv_t[ic][:], wv_d[l, ic * 128 : (ic + 1) * 128, :]
                    )
                for tb in range(TB):
                    vs = stg.tile([128, VW], BF16, tag="vst", name=f"vs{l}_{tb}")
                    vsv = vs[:].rearrange("p (n e) -> p n e", e=DH + 1)
                    for n0, nn in _ns(Hd, 512):
                        ps = psC.tile([128, nn], F32, tag="ctx", name=f"psv{l}_{tb}_{n0}")
                        nc.tensor.matmul(
                            ps[:],
                            ones_r[0:1, 0:128],
                            bv_t[0:1, n0 : n0 + nn],
                            start=True,
                            stop=False,
                        )
                        for ic in range(HC):
                            nc.tensor.matmul(
                                ps[:],
                                h_bf[ic][:, tb * 128 : (tb + 1) * 128],
                                wv_t[ic][:, n0 : n0 + nn],
                                start=False,
                                stop=(ic == HC - 1),
                            )
                        h0, hn = n0 // DH, nn // DH  # head range of this chunk
                        nc.vector.tensor_copy(
                            vsv[:, h0 : h0 + hn, 0:DH],
                            ps[:].rearrange("p (n d) -> p n d", d=DH),
                        )
                    nc.vector.memset(vsv[:, :, DH : DH + 1], 1.0)
                    nc.sync.dma_start(kvi_V[tb * 128 : (tb + 1) * 128, :], vs[:])

                # ---- AllGather K/V within the sample's core group ----
                _ph('AllGather', l, nc)
                if fake_cc:
                    for half in range(NHALF):
                        nc.sync.dma_start(kv_out[half], kv_in[:])
                else:
                    nc.gpsimd.collective_compute(
                        "AllGather",
                        AluOpType.bypass,
                        replica_groups=groups,
                        ins=[kv_in[:].opt()],
                        outs=[kv_out[:].opt()],
                    )

                # ---- Q projection (overlaps the collective) ----
                _ph('Q_proj', l, nc)
                wq_t = [
                    wp.tile([128, Hd], BF16, tag="w", name=f"wq{l}_{ic}")
                    for ic in range(HC)
                ]
                for ic in range(HC):
                    nc.sync.dma_start(
                        wq_t[ic][:], wq_d[l, ic * 128 : (ic + 1) * 128, :]
                    )
                for hc in range(HC):
                    for q0, qn in QS:
                        ps = psC.tile([128, qn], F32, tag="ctx", name=f"psq{l}_{hc}_{q0}")
                        for ic in range(HC):
                            nc.tensor.matmul(
                                ps[:],
                                wq_t[ic][:, hc * 128 : (hc + 1) * 128],
                                h_bf[ic][:, q0 : q0 + qn],
                                start=(ic == 0),
                                stop=(ic == HC - 1),
                            )
                        nc.vector.tensor_scalar_add(
                            qT[hc][:, q0 : q0 + qn], ps[:],
                            par_t[:, O_BQ + hc : O_BQ + hc + 1],
                        )

                # ---- unpack gathered K/V ----
                _ph('unpack_KV', l, nc)
                for half in range(NHALF):
                    kvo_K = kv_out[half, 0 : Hd * TOK].rearrange("(h t) -> h t", t=TOK)
                    kvo_V = kv_out[half, Hd * TOK :].rearrange("(t w) -> t w", w=VW)
                    for hc in range(HC):
                        nc.sync.dma_start(
                            kT[half][hc][:], kvo_K[hc * 128 : (hc + 1) * 128, :]
                        )
                    for tb in range(TPH):
                        kb = half * TPH + tb
                        nc.sync.dma_start(
                            v_sb[kb][:], kvo_V[tb * 128 : (tb + 1) * 128, :]
                        )

                # ---- per q-half: attention -> O -> LN1 -> FFN -> LN2 ----
                def layer_norm(q0, qn, w_off, b_off, tag):
                    # squares scratch lives in qT columns of this half (dead
                    # between this half's attention and next-layer Q proj)
                    for hc in range(HC):
                        nc.vector.tensor_copy(
                            h_bf[hc][:, q0 : q0 + qn], lnin[hc][:, q0 : q0 + qn]
                        )
                        # squares from the bf16 copy: all-bf16 SBUF operands
                        # let the DVE run in its 2x mode
                        nc.vector.tensor_mul(
                            qT[hc][:, q0 : q0 + qn],
                            h_bf[hc][:, q0 : q0 + qn],
                            h_bf[hc][:, q0 : q0 + qn],
                        )
                    s_ps = psW.tile([1, qn], F32, tag="w512", name=f"sps{tag}{l}_{q0}")
                    q_ps = psW.tile([1, qn], F32, tag="w512", name=f"qps{tag}{l}_{q0}")
                    for hc in range(HC):
                        nc.tensor.matmul(
                            s_ps[:],
                            ones_c[:],
                            h_bf[hc][:, q0 : q0 + qn],
                            start=(hc == 0),
                            stop=(hc == HC - 1),
                        )
                    for hc in range(HC):
                        nc.tensor.matmul(
                            q_ps[:],
                            ones_c[:],
                            qT[hc][:, q0 : q0 + qn],
                            start=(hc == 0),
                            stop=(hc == HC - 1),
                        )
                    t1 = rowf.tile([1, qn], F32, tag="t1", bufs=1, name=f"t1{tag}{l}_{q0}")
                    nc.scalar.square(t1[:], s_ps[:])
                    nc.vector.scalar_tensor_tensor(
                        t1[:],
                        t1[:],
                        -1.0 / Hd,
                        q_ps[:],
                        op0=AluOpType.mult,
                        op1=AluOpType.add,
                    )
                    # t1 = sumsq - sum^2/Hd ; rstd = 1/sqrt(t1/Hd + eps)
                    nc.scalar.activation(
                        t1[:], t1[:], AF.Sqrt, scale=1.0 / Hd, bias=eps_sb[0:1, :]
                    )
                    nc.vector.reciprocal(t1[:], t1[:])
                    mr_b = rowb.tile(
                        [1, 2 * qn], BF16, tag="mrb", bufs=1, name=f"mr{tag}{l}_{q0}"
                    )
                    nc.vector.tensor_scalar_mul(mr_b[:, 0:qn], s_ps[:], 1.0 / Hd)
                    nc.vector.tensor_copy(mr_b[:, qn : 2 * qn], t1[:])
                    bpool, btag = (psS, "sc") if q0 >= 512 else (psW, "w512")
                    m_bc = bpool.tile([128, qn], F32, tag=btag, name=f"mbc{tag}{l}_{q0}")
                    r_bc = psW.tile([128, qn], F32, tag="w512", name=f"rbc{tag}{l}_{q0}")
                    nc.tensor.matmul(
                        m_bc[:], ones_r[0:1, 0:128], mr_b[0:1, 0:qn],
                        start=True, stop=True,
                    )
                    nc.tensor.matmul(
                        r_bc[:], ones_r[0:1, 0:128], mr_b[0:1, qn : 2 * qn],
                        start=True, stop=True,
                    )
                    for hc in range(HC):
                        t = scr.tile([128, qn], F32, tag="scr", name=f"sc{tag}{l}_{q0}_{hc}")
                        nc.vector.tensor_sub(t[:], lnin[hc][:, q0 : q0 + qn], m_bc[:])
                        nc.vector.tensor_mul(t[:], t[:], r_bc[:])
                        nc.vector.tensor_scalar(
                            h_m[hc][:, q0 : q0 + qn],
                            t[:],
                            par_t[:, w_off + hc : w_off + hc + 1],
                            par_t[:, b_off + hc : b_off + hc + 1],
                            AluOpType.mult,
                            AluOpType.add,
                        )
                        nc.vector.tensor_copy(
                            h_bf[hc][:, q0 : q0 + qn], h_m[hc][:, q0 : q0 + qn]
                        )

                # O weights loaded once per layer, shared by both q-halves
                wo_t = [
                    wp.tile([128, Hd], BF16, tag="w", name=f"wo{l}_{ic}")
                    for ic in range(HC)
                ]
                for ic in range(HC):
                    nc.sync.dma_start(
                        wo_t[ic][:], wo_d[l, ic * 128 : (ic + 1) * 128, :]
                    )

                for qi, (q0, qn) in enumerate(QS):
                    # ---- attention for this q-half ----
                    _ph(f'attn{qi}', l, nc)
                    for hp in range(HC):
                        ctx = [
                            psC.tile([65, qn], F32, tag="ctx", name=f"ctx{l}_{qi}_{hp}_{p}")
                            for p in range(2)
                        ]
                        for kb in range(KB):
                            half, tb = kb // TPH, kb % TPH
                            sc = psS.tile(
                                [128, 2 * qn], F32, tag="sc", name=f"sc{l}_{qi}_{hp}_{kb}"
                            )
                            for par_i in range(2):
                                b0 = 64 * par_i
                                nc.tensor.matmul(
                                    sc[:, par_i * qn : par_i * qn + qn],
                                    kT[half][hp][b0 : b0 + 64, tb * 128 : (tb + 1) * 128],
                                    qT[hp][b0 : b0 + 64, q0 : q0 + qn],
                                    start=True,
                                    stop=True,
                                    tile_position=(b0, 0),
                                )
                            ex = expp.tile(
                                [128, 2 * qn], BF16, tag="exp", name=f"ex{l}_{qi}_{hp}_{kb}"
                            )
                            nc.scalar.activation(
                                ex[:],
                                sc[:],
                                AF.Exp,
                                bias=mask_sb[:, kb : kb + 1],
                                scale=float(1.0 / np.sqrt(DH)),
                            )
                            for par_i in range(2):
                                head = 2 * hp + par_i
                                nc.tensor.matmul(
                                    ctx[par_i][:, :],
                                    v_sb[kb][:, head * (DH + 1) : (head + 1) * (DH + 1)],
                                    ex[:, par_i * qn : par_i * qn + qn],
                                    start=(kb == 0),
                                    stop=(kb == KB - 1),
                                )
                        # normalize: ctx[0:64] * (1/sumexp) broadcast over partitions
                        for par_i in range(2):
                            rec_b = rowb.tile(
                                [1, qn], BF16, tag="recb", name=f"rb{l}_{qi}_{hp}_{par_i}"
                            )
                            with nc.allow_low_precision("softmax denom in bf16"):
                                nc.vector.reciprocal(rec_b[:], ctx[par_i][64:65, :])
                            bpool2, btag2 = (psW, "w512") if qi == 0 else (psS, "sc")
                            bc = bpool2.tile(
                                [64, qn], F32, tag=btag2, name=f"bc{l}_{qi}_{hp}_{par_i}"
                            )
                            nc.tensor.matmul(
                                bc[:],
                                ones_r[0:1, 0:64],
                                rec_b[0:1, :],
                                start=True,
                                stop=True,
                            )
                            bcs = expp.tile(
                                [64, qn], BF16, tag="bcs", bufs=2, name=f"bcs{l}_{qi}_{hp}_{par_i}"
                            )
                            nc.vector.tensor_copy(bcs[:], bc[:])
                            b0 = 64 * par_i
                            nc.vector.tensor_tensor(
                                ctxT[hp][b0 : b0 + 64, q0 : q0 + qn],
                                ctx[par_i][0:64, :],
                                bcs[:],
                                op=AluOpType.mult,
                            )

                    # ---- O projection + residual -> lnin ----
                    _ph(f'O_proj{qi}', l, nc)
                    for hc in range(HC):
                        wpool, wtag = (psS, "sc") if qi == 1 and hc % 2 else (psW, "w512")
                        ps = wpool.tile([128, qn], F32, tag=wtag, name=f"pso{l}_{qi}_{hc}")
                        for ic in range(HC):
                            nc.tensor.matmul(
                                ps[:],
                                wo_t[ic][:, hc * 128 : (hc + 1) * 128],
                                ctxT[ic][:, q0 : q0 + qn],
                                start=(ic == 0),
                                stop=(ic == HC - 1),
                            )
                        nc.vector.scalar_tensor_tensor(
                            lnin[hc][:, q0 : q0 + qn],
                            ps[:],
                            par_t[:, O_BO + hc : O_BO + hc + 1],
                            h_m[hc][:, q0 : q0 + qn],
                            op0=AluOpType.add,
                            op1=AluOpType.add,
                        )

                    _ph(f'LN1_{qi}', l, nc)
                    layer_norm(q0, qn, O_L1W, O_L1B, "a")

                    # ---- FFN for this q-half ----
                    _ph(f'FFN{qi}', l, nc)
                    for oc in range(FC):
                        if oc % 2 == 0:
                            wi_t = wip.tile(
                                [128, 2 * Hd], BF16, tag="wi", name=f"wi{l}_{qi}_{oc}"
                            )
                            nc.sync.dma_start(wi_t[:], wi_d[l, oc // 2])
                        ob = (oc % 2) * Hd
                        # the trailing half's FFN runs after attention is done,
                        # so the idle score banks double as extra accumulators
                        wpool, wtag = (psS, "sc") if qi == 1 and oc % 2 else (psW, "w512")
                        ps = wpool.tile([128, qn], F32, tag=wtag, name=f"psf{l}_{qi}_{oc}")
                        for ic in range(HC):
                            nc.tensor.matmul(
                                ps[:],
                                wi_t[:, ob + ic * 128 : ob + (ic + 1) * 128],
                                h_bf[ic][:, q0 : q0 + qn],
                                start=(ic == 0),
                                stop=(ic == HC - 1),
                            )
                        nc.scalar.activation(
                            ffT(oc, q0)[:, 0:qn], ps[:], AF.Gelu,
                            bias=bi_t[:, oc : oc + 1],
                        )
                    for hc in range(HC):
                        wf_t = wfp.tile([128, FF], BF16, tag="wf", name=f"wf{l}_{qi}_{hc}")
                        nc.sync.dma_start(wf_t[:], wf_d[l, hc])
                        wpool, wtag = (psS, "sc") if qi == 1 and hc % 2 else (psW, "w512")
                        ps = wpool.tile([128, qn], F32, tag=wtag, name=f"psg{l}_{qi}_{hc}")
                        for fc in range(FC):
                            nc.tensor.matmul(
                                ps[:],
                                wf_t[:, fc * 128 : (fc + 1) * 128],
                                ffT(fc, q0)[:, 0:qn],
                                start=(fc == 0),
                                stop=(fc == FC - 1),
                            )
                        nc.vector.scalar_tensor_tensor(
                            lnin[hc][:, q0 : q0 + qn],
                            ps[:],
                            par_t[:, O_BF + hc : O_BF + hc + 1],
                            h_m[hc][:, q0 : q0 + qn],
                            op0=AluOpType.add,
                            op1=AluOpType.add,
                        )

                    _ph(f'LN2_{qi}', l, nc)
                    layer_norm(q0, qn, O_L2W, O_L2B, "b")

            _ph('output', 99, nc)
            # ------------- output (transpose back to token-major) -------------
            for tb in range(TB):
                ysb = scr.tile([128, Hd], F32, tag="scr", name=f"ysb{tb}")
                for hc in range(HC):
                    tpool, ttag = (psS, "sc") if hc % 2 else (psW, "w512")
                    pst = tpool.tile([128, 128], F32, tag=ttag, name=f"yp{tb}_{hc}")
                    nc.tensor.transpose(
                        pst[:], h_m[hc][:, tb * 128 : (tb + 1) * 128], ident[:]
                    )
                    nc.vector.tensor_copy(ysb[:, hc * 128 : (hc + 1) * 128], pst[:])
                nc.sync.dma_start(y_d[tb * 128 : (tb + 1) * 128, :], ysb[:])

        for fr in reversed(_frees):
            fr()

    nc.compile()
    return nc


# ---------------------------------------------------------------------------
# host-side prep + execution
# ---------------------------------------------------------------------------


def prep_shared_inputs(cfg: Cfg, d: dict) -> dict:
    """Inputs identical on every core (weights)."""
    L, Hd, FF, HC, FC = cfg.L, cfg.Hd, cfg.FF, cfg.HC, cfg.FC

    def colpack(x, n):  # [L, n*128] -> [L, 128, n]
        return np.ascontiguousarray(
            np.asarray(x, np.float32).reshape(L, n, 128).transpose(0, 2, 1)
        )

    par = np.concatenate(
        [
            colpack(d["bq"], HC),
            colpack(d["bk"], HC),
            colpack(d["bo"], HC),
            colpack(d["bf"], HC),
            colpack(d["ln1_w"], HC),
            colpack(d["ln1_b"], HC),
            colpack(d["ln2_w"], HC),
            colpack(d["ln2_b"], HC),
        ],
        axis=2,
    )
    # wi[l, oc, p, ic*128+j] = Wi[l, ic*128+p, oc*128+j], then 4 oc per DMA
    # row-block: wi4[l, g, p, m*Hd + ic*128+j] = wi[l, 4g+m, p, ic*128+j]
    wi_r = np.ascontiguousarray(
        np.asarray(d["Wi"], np.float32)
        .reshape(L, HC, 128, FC, 128)
        .transpose(0, 3, 2, 1, 4)
        .reshape(L, FC // 2, 2, 128, Hd)
        .transpose(0, 1, 3, 2, 4)
        .reshape(L, FC // 2, 128, 2 * Hd)
        .astype(BF)
    )
    # wf[l, oc2, p, fc*128+j] = Wf[l, fc*128+p, oc2*128+j]
    wf_r = np.ascontiguousarray(
        np.asarray(d["Wf"], np.float32)
        .reshape(L, FC, 128, HC, 128)
        .transpose(0, 3, 2, 1, 4)
        .reshape(L, HC, 128, FF)
        .astype(BF)
    )
    return {
        "wq": np.asarray(d["Wq"], np.float32).astype(BF),
        "wk": np.asarray(d["Wk"], np.float32).astype(BF),
        "wv": np.asarray(d["Wv"], np.float32).astype(BF),
        "wo": np.asarray(d["Wo"], np.float32).astype(BF),
        "wi": wi_r,
        "wf": wf_r,
        "par": par,
        "bi": colpack(d["bi"], FC),
        "bv": np.asarray(d["bv"], np.float32).astype(BF)[:, None, :],
        "wemb": np.asarray(d["word_emb"], np.float32).astype(BF),
        "lne": np.concatenate(
            [
                np.asarray(d["ln_e_w"], np.float32).reshape(HC, 128).T,
                np.asarray(d["ln_e_b"], np.float32).reshape(HC, 128).T,
            ],
            axis=1,
        ),
    }


def prep_core_inputs(cfg: Cfg, core: int, d: dict, shared: dict) -> dict:
    TOK, TB, KB = cfg.TOK, cfg.TB, cfg.KB
    b, hh = core // cfg.NHALF, core % cfg.NHALF
    ids = np.asarray(d["input_ids"], np.int32)[b, hh * TOK : (hh + 1) * TOK]
    mask = np.asarray(d["attention_mask"], np.float32)[b, 0, 0, :]
    pos = (
        np.asarray(d["pos_emb"], np.float32)[hh * TOK : (hh + 1) * TOK]
        + np.asarray(d["type_emb"], np.float32)[0][None, :]
    )
    m = dict(shared)
    m["pos"] = np.ascontiguousarray(pos, dtype=np.float32)
    m["ids"] = np.ascontiguousarray(ids.reshape(TB, 128).T)
    m["mask"] = np.ascontiguousarray(mask.reshape(KB, 128).T)
    return m


_CACHE: dict = {}


def kernel(**inputs) -> np.ndarray:
    cfg = Cfg()
    B = inputs["input_ids"].shape[0]
    if "nc" not in _CACHE:
        _CACHE["nc"] = build(cfg)
    nc = _CACHE["nc"]
    shared = prep_shared_inputs(cfg, inputs)
    in_maps = [prep_core_inputs(cfg, c, inputs, shared) for c in range(cfg.n_cores)]
    res = run_bass_kernel_spmd(nc, in_maps, core_ids=list(range(cfg.n_cores)))
    out = np.zeros((B, cfg.S, cfg.Hd), np.float32)
    for c in range(cfg.n_cores):
        b, hh = c // cfg.NHALF, c % cfg.NHALF
        out[b, hh * cfg.TOK : (hh + 1) * cfg.TOK, :] = res.results[c]["y"]
    return out



# revision 15
# speedup vs baseline: 1.5836x; 1.0421x over previous
"""BERT-base encoder (12L, B=4, S=2048, H=768) on 8 Trainium2 NeuronCores.

Sharding: 8 shards of 1024 tokens each (sample b = core//2, seq-half = core%2).
Per layer, each core computes K/V for its own 1024 tokens and AllGathers them
within the core pair owning the sample, so attention sees the full 2048-token
context while all other work stays perfectly data-parallel.

On-chip layout: activations are kept feature-major (h^T, [768, 1024]) so every
projection is a plain accumulated matmul with the stored [in, out] weights as
the stationary operand. Attention computes scores^T ([k, q]) so the
probs @ V matmul needs no transposes; softmax normalization uses a ones-column
appended to V (row 64 of the ctx PSUM accumulates sum(exp)) plus a K=1
ones-matmul to broadcast 1/sumexp across partitions. LayerNorm statistics are
computed with ones-vector matmuls (sums over the partition dim land in PSUM).

Everything after the K/V AllGather is processed in two independent 512-token
q-halves (attention -> O -> LN1 -> FFN -> LN2 per half), which lets the Tile
scheduler overlap the scalar-engine-bound softmax exp of one half with the
PE-bound FFN of the other, and the next layer's K/V projections with the
trailing half's FFN.
"""

import dataclasses

import numpy as np
import ml_dtypes

import concourse.bass as bass
import concourse.tile as tile
from concourse import bacc, mybir
from concourse.bass import IndirectOffsetOnAxis
from concourse.bass_utils import run_bass_kernel_spmd
from concourse.masks import make_identity
from concourse.alu_op_type import AluOpType

F32 = mybir.dt.float32
BF16 = mybir.dt.bfloat16
I32 = mybir.dt.int32
AF = mybir.ActivationFunctionType
BF = ml_dtypes.bfloat16

PHASE_HOOK = None  # optional (tag, layer, nc) callback for profiling builds


def _ph(tag, l, nc):
    if PHASE_HOOK is not None:
        PHASE_HOOK(tag, l, nc)


@dataclasses.dataclass
class Cfg:
    L: int = 12
    NH: int = 12
    DH: int = 64
    FF: int = 3072
    V: int = 30522
    TOK: int = 1024  # tokens per core
    S: int = 2048  # full sequence
    n_cores: int = 8
    EPS: float = 1e-12

    @property
    def Hd(self):
        return self.NH * self.DH

    @property
    def HC(self):
        return self.Hd // 128  # hidden chunks == head pairs

    @property
    def TB(self):
        return self.TOK // 128

    @property
    def KB(self):
        return self.S // 128

    @property
    def FC(self):
        return self.FF // 128

    @property
    def NHALF(self):
        return self.S // self.TOK  # cores per sample

    @property
    def QS(self):
        return [(q, min(512, self.TOK - q)) for q in range(0, self.TOK, 512)]


def _ns(total, size):
    return [(s, min(size, total - s)) for s in range(0, total, size)]


def build(cfg: Cfg, fake_cc: bool = False):
    L, NH, DH, FF, V = cfg.L, cfg.NH, cfg.DH, cfg.FF, cfg.V
    TOK, S, Hd = cfg.TOK, cfg.S, cfg.Hd
    HC, TB, KB, FC, NHALF = cfg.HC, cfg.TB, cfg.KB, cfg.FC, cfg.NHALF
    QS = cfg.QS
    TPH = TOK // 128  # k blocks per gathered half

    nc = bacc.Bacc(
        "TRN2",
        target_bir_lowering=False,
        debug=False,
        enable_asserts=True,
        num_devices=cfg.n_cores,
    )

    # ---------------- DRAM I/O ----------------
    wq_d = nc.dram_tensor("wq", [L, Hd, Hd], BF16, kind="ExternalInput").ap()
    wk_d = nc.dram_tensor("wk", [L, Hd, Hd], BF16, kind="ExternalInput").ap()
    wv_d = nc.dram_tensor("wv", [L, Hd, Hd], BF16, kind="ExternalInput").ap()
    wo_d = nc.dram_tensor("wo", [L, Hd, Hd], BF16, kind="ExternalInput").ap()
    # wi batched 2 output-chunks per DMA row-block: [L, FC//2, 128, 2*Hd]
    wi_d = nc.dram_tensor("wi", [L, FC // 2, 128, 2 * Hd], BF16, kind="ExternalInput").ap()
    wf_d = nc.dram_tensor("wf", [L, HC, 128, FF], BF16, kind="ExternalInput").ap()
    # packed per-layer params: bq|bk|bo|bf|l1w|l1b|l2w|l2b, each HC cols
    par_d = nc.dram_tensor("par", [L, 128, 8 * HC], F32, kind="ExternalInput").ap()
    bi_d = nc.dram_tensor("bi", [L, 128, FC], F32, kind="ExternalInput").ap()
    bv_d = nc.dram_tensor("bv", [L, 1, Hd], BF16, kind="ExternalInput").ap()
    we_d = nc.dram_tensor("wemb", [V, Hd], BF16, kind="ExternalInput").ap()
    pos_d = nc.dram_tensor("pos", [TOK, Hd], F32, kind="ExternalInput").ap()
    # embedding LN gain|bias packed column-wise: [128, w0..w5 b0..b5]
    lne_d = nc.dram_tensor("lne", [128, 2 * HC], F32, kind="ExternalInput").ap()
    ids_d = nc.dram_tensor("ids", [128, TB], I32, kind="ExternalInput").ap()
    mask_d = nc.dram_tensor("mask", [128, KB], F32, kind="ExternalInput").ap()
    y_d = nc.dram_tensor("y", [TOK, Hd], F32, kind="ExternalOutput").ap()

    VW = NH * (DH + 1)  # V row width incl. interleaved ones columns
    KVN = Hd * TOK + TOK * VW
    kv_in = nc.dram_tensor("kv_in", [KVN], BF16, kind="Internal").ap()
    kv_out = nc.dram_tensor("kv_out", [NHALF, KVN], BF16, kind="Internal").ap()
    kvi_K = kv_in[0 : Hd * TOK].rearrange("(h t) -> h t", t=TOK)
    kvi_V = kv_in[Hd * TOK :].rearrange("(t w) -> t w", w=VW)

    groups = [
        [g * NHALF + i for i in range(NHALF)] for g in range(cfg.n_cores // NHALF)
    ]

    with tile.TileContext(nc) as tc:
        # ---------------- persistent SBUF ----------------
        _frees = []  # keep pool-release closures alive for the whole build

        def single(name, shape, dtype):
            t, fr = tc.tile(shape, dtype, name=name)
            _frees.append(fr)
            return t

        h_m = [single(f"h_m{i}", [128, TOK], F32) for i in range(HC)]
        lnin = [single(f"lnin{i}", [128, TOK], F32) for i in range(HC)]
        h_bf = [single(f"h_bf{i}", [128, TOK], BF16) for i in range(HC)]
        qT = [single(f"qT{i}", [128, TOK], BF16) for i in range(HC)]
        kT = [
            [single(f"kT{h}_{i}", [128, TOK], BF16) for i in range(HC)]
            for h in range(NHALF)
        ]
        v_sb = [single(f"v_sb{i}", [128, NH * (DH + 1)], BF16) for i in range(KB)]
        ctxT = [single(f"ctxT{i}", [128, TOK], BF16) for i in range(HC)]
        # half of the FFN activation slots; the rest alias qT/ctxT columns of
        # the half currently in its FFN phase (dead there, live for the other)
        ffx = [single(f"ffx{i}", [128, 512], BF16) for i in range(FC - 2 * HC)]

        ids_sb = single("ids_sb", [128, TB], I32)
        mask_sb = single("mask_sb", [128, KB], F32)
        ones_c = single("ones_c", [128, 1], BF16)  # stats lhsT
        ones_r = single("ones_r", [1, 128], BF16)  # broadcast lhsT
        ident = single("ident", [128, 128], F32)
        eps_sb = single("eps_sb", [128, 1], F32)
        nc.vector.memset(eps_sb[:], 1e-12)

        nc.vector.memset(ones_c[:], 1.0)
        nc.vector.memset(ones_r[:], 1.0)
        make_identity(nc, ident[:])
        nc.sync.dma_start(ids_sb[:], ids_d[:, :])
        nc.sync.dma_start(mask_sb[:], mask_d[:, :])

        def ffT(oc, q0):
            """[128, 512] bf16 slot for FFN activation block oc of the q-half
            starting at column q0."""
            if oc < len(ffx):
                return ffx[oc][:, 0:512]
            oc -= len(ffx)
            if oc < HC:
                return qT[oc][:, q0 : q0 + 512]
            return ctxT[oc - HC][:, q0 : q0 + 512]

        with (
            tc.tile_pool(name="wp", bufs=9) as wp,
            tc.tile_pool(name="wip", bufs=3) as wip,
            tc.tile_pool(name="wfp", bufs=2) as wfp,
            tc.tile_pool(name="expp", bufs=4) as expp,
            tc.tile_pool(name="scr", bufs=2) as scr,
            tc.tile_pool(name="rowf", bufs=2) as rowf,
            tc.tile_pool(name="rowb", bufs=2) as rowb,
            tc.tile_pool(name="stg", bufs=2) as stg,
            tc.tile_pool(name="parp", bufs=2) as parp,
            tc.tile_pool(name="psS", bufs=2, space="PSUM") as psS,  # [128,1024] 2 banks
            tc.tile_pool(name="psC", bufs=2, space="PSUM") as psC,  # [65,512] ctx accum
            tc.tile_pool(name="psW", bufs=2, space="PSUM") as psW,  # [128,512] work
        ):
            # ---------------- embedding ----------------
            _ph('embed', -1, nc)
            lne_t, fr1 = tc.tile([128, 2 * HC], F32, name="lne_t")
            nc.sync.dma_start(lne_t[:], lne_d[:, :])

            bn_sub = 256 if Hd % 256 == 0 else 128
            nsub = Hd // bn_sub
            for tb in range(TB):
                emb_g = stg.tile([128, Hd], BF16, tag="vst", name=f"embg{tb}")
                nc.gpsimd.indirect_dma_start(
                    out=emb_g[:],
                    out_offset=None,
                    in_=we_d[:, :],
                    in_offset=IndirectOffsetOnAxis(ap=ids_sb[:, tb : tb + 1], axis=0),
                )
                pos_t = scr.tile([128, Hd], F32, tag="scr", name=f"pos{tb}")
                nc.sync.dma_start(pos_t[:], pos_d[tb * 128 : (tb + 1) * 128, :])
                x = scr.tile([128, Hd], F32, tag="scr", name=f"embx{tb}")
                nc.vector.tensor_copy(x[:], emb_g[:])
                nc.vector.tensor_add(x[:], x[:], pos_t[:])
                # LN over the free (feature) dim
                stats = scr.tile([128, nsub, 6], F32, tag="bst", name=f"bst{tb}")
                for sgi in range(nsub):
                    nc.vector.bn_stats(
                        stats[:, sgi, :], x[:, sgi * bn_sub : (sgi + 1) * bn_sub]
                    )
                mv = scr.tile([128, 2], F32, tag="bmv", name=f"bmv{tb}")
                nc.vector.bn_aggr(mv[:], stats[:])
                rstd = scr.tile([128, 1], F32, tag="brs", name=f"brs{tb}")
                nc.scalar.activation(rstd[:], mv[:, 1:2], AF.Sqrt, bias=eps_sb[:])
                nc.vector.reciprocal(rstd[:], rstd[:])
                nc.vector.tensor_scalar(
                    x[:], x[:], mv[:, 0:1], rstd[:], AluOpType.subtract, AluOpType.mult
                )
                # transpose to feature-major, then gain/bias per feature chunk
                for hc in range(HC):
                    tpool, ttag = (psS, "sc") if hc % 2 else (psW, "w512")
                    pst = tpool.tile([128, 128], F32, tag=ttag, name=f"tp{tb}_{hc}")
                    nc.tensor.transpose(
                        pst[:], x[:, hc * 128 : (hc + 1) * 128], ident[:]
                    )
                    nc.vector.tensor_scalar(
                        h_m[hc][:, tb * 128 : (tb + 1) * 128],
                        pst[:],
                        lne_t[:, hc : hc + 1],
                        lne_t[:, HC + hc : HC + hc + 1],
                        AluOpType.mult,
                        AluOpType.add,
                    )
                    nc.vector.tensor_copy(
                        h_bf[hc][:, tb * 128 : (tb + 1) * 128],
                        h_m[hc][:, tb * 128 : (tb + 1) * 128],
                    )
            fr1()

            # ---------------- layers ----------------
            for l in range(L):
                par_t = parp.tile([128, 8 * HC], F32, tag="par", name=f"par{l}")
                nc.sync.dma_start(par_t[:], par_d[l])
                bi_t = parp.tile([128, FC], F32, tag="bi", name=f"bi{l}")
                nc.sync.dma_start(bi_t[:], bi_d[l])
                bv_t = parp.tile([1, Hd], BF16, tag="bv", bufs=1, name=f"bv{l}")
                nc.sync.dma_start(bv_t[:], bv_d[l])
                O_BQ, O_BK, O_BO, O_BF = 0, HC, 2 * HC, 3 * HC
                O_L1W, O_L1B, O_L2W, O_L2B = 4 * HC, 5 * HC, 6 * HC, 7 * HC

                # ---- K projection -> kv staging ----
                _ph('K_proj', l, nc)
                wk_t = [
                    wp.tile([128, Hd], BF16, tag="w", name=f"wk{l}_{ic}")
                    for ic in range(HC)
                ]
                for ic in range(HC):
                    nc.sync.dma_start(
                        wk_t[ic][:], wk_d[l, ic * 128 : (ic + 1) * 128, :]
                    )
                for hc in range(HC):
                    # stage K through ctxT[hc], dead since last layer's O proj
                    ks = ctxT[hc]
                    for q0, qn in QS:
                        ps = psC.tile([128, qn], F32, tag="ctx", name=f"psk{l}_{hc}_{q0}")
                        for ic in range(HC):
                            nc.tensor.matmul(
                                ps[:],
                                wk_t[ic][:, hc * 128 : (hc + 1) * 128],
                                h_bf[ic][:, q0 : q0 + qn],
                                start=(ic == 0),
                                stop=(ic == HC - 1),
                            )
                        nc.vector.tensor_scalar_add(
                            ks[:, q0 : q0 + qn], ps[:], par_t[:, O_BK + hc : O_BK + hc + 1]
                        )
                    nc.sync.dma_start(kvi_K[hc * 128 : (hc + 1) * 128, :], ks[:])

                # ---- V projection (token-major, bias via K=1 ones matmul) ----
                _ph('V_proj', l, nc)
                wv_t = [
                    wp.tile([128, Hd], BF16, tag="w", name=f"wv{l}_{ic}")
                    for ic in range(HC)
                ]
                for ic in range(HC):
                    nc.sync.dma_start(
                        wv_t[ic][:], wv_d[l, ic * 128 : (ic + 1) * 128, :]
                    )
                for tb in range(TB):
                    vs = stg.tile([128, VW], BF16, tag="vst", name=f"vs{l}_{tb}")
                    vsv = vs[:].rearrange("p (n e) -> p n e", e=DH + 1)
                    for n0, nn in _ns(Hd, 512):
                        ps = psC.tile([128, nn], F32, tag="ctx", name=f"psv{l}_{tb}_{n0}")
                        nc.tensor.matmul(
                            ps[:],
                            ones_r[0:1, 0:128],
                            bv_t[0:1, n0 : n0 + nn],
                            start=True,
                            stop=False,
                        )
                        for ic in range(HC):
                            nc.tensor.matmul(
                                ps[:],
                                h_bf[ic][:, tb * 128 : (tb + 1) * 128],
                                wv_t[ic][:, n0 : n0 + nn],
                                start=False,
                                stop=(ic == HC - 1),
                            )
                        h0, hn = n0 // DH, nn // DH  # head range of this chunk
                        nc.vector.tensor_copy(
                            vsv[:, h0 : h0 + hn, 0:DH],
                            ps[:].rearrange("p (n d) -> p n d", d=DH),
                        )
                    nc.vector.memset(vsv[:, :, DH : DH + 1], 1.0)
                    nc.sync.dma_start(kvi_V[tb * 128 : (tb + 1) * 128, :], vs[:])

                # ---- AllGather K/V within the sample's core group ----
                _ph('AllGather', l, nc)
                if fake_cc:
                    for half in range(NHALF):
                        nc.sync.dma_start(kv_out[half], kv_in[:])
                else:
                    nc.gpsimd.collective_compute(
                        "AllGather",
                        AluOpType.bypass,
                        replica_groups=groups,
                        ins=[kv_in[:].opt()],
                        outs=[kv_out[:].opt()],
                    )

                # ---- Q projection (overlaps the collective) ----
                _ph('Q_proj', l, nc)
                wq_t = [
                    wp.tile([128, Hd], BF16, tag="w", name=f"wq{l}_{ic}")
                    for ic in range(HC)
                ]
                for ic in range(HC):
                    nc.sync.dma_start(
                        wq_t[ic][:], wq_d[l, ic * 128 : (ic + 1) * 128, :]
                    )
                for hc in range(HC):
                    for q0, qn in QS:
                        ps = psC.tile([128, qn], F32, tag="ctx", name=f"psq{l}_{hc}_{q0}")
                        for ic in range(HC):
                            nc.tensor.matmul(
                                ps[:],
                                wq_t[ic][:, hc * 128 : (hc + 1) * 128],
                                h_bf[ic][:, q0 : q0 + qn],
                                start=(ic == 0),
                                stop=(ic == HC - 1),
                            )
                        nc.vector.tensor_scalar_add(
                            qT[hc][:, q0 : q0 + qn], ps[:],
                            par_t[:, O_BQ + hc : O_BQ + hc + 1],
                        )

                # ---- unpack gathered K/V ----
                _ph('unpack_KV', l, nc)
                for half in range(NHALF):
                    kvo_K = kv_out[half, 0 : Hd * TOK].rearrange("(h t) -> h t", t=TOK)
                    kvo_V = kv_out[half, Hd * TOK :].rearrange("(t w) -> t w", w=VW)
                    for hc in range(HC):
                        nc.sync.dma_start(
                            kT[half][hc][:], kvo_K[hc * 128 : (hc + 1) * 128, :]
                        )
                    for tb in range(TPH):
                        kb = half * TPH + tb
                        nc.sync.dma_start(
                            v_sb[kb][:], kvo_V[tb * 128 : (tb + 1) * 128, :]
                        )

                # ---- per q-half: attention -> O -> LN1 -> FFN -> LN2 ----
                def layer_norm(q0, qn, w_off, b_off, tag):
                    # squares scratch lives in qT columns of this half (dead
                    # between this half's attention and next-layer Q proj)
                    for hc in range(HC):
                        nc.vector.tensor_copy(
                            h_bf[hc][:, q0 : q0 + qn], lnin[hc][:, q0 : q0 + qn]
                        )
                        # squares from the bf16 copy: all-bf16 SBUF operands
                        # let the DVE run in its 2x mode
                        nc.vector.tensor_mul(
                            qT[hc][:, q0 : q0 + qn],
                            h_bf[hc][:, q0 : q0 + qn],
                            h_bf[hc][:, q0 : q0 + qn],
                        )
                    s_ps = psW.tile([1, qn], F32, tag="w512", name=f"sps{tag}{l}_{q0}")
                    q_ps = psW.tile([1, qn], F32, tag="w512", name=f"qps{tag}{l}_{q0}")
                    for hc in range(HC):
                        nc.tensor.matmul(
                            s_ps[:],
                            ones_c[:],
                            h_bf[hc][:, q0 : q0 + qn],
                            start=(hc == 0),
                            stop=(hc == HC - 1),
                        )
                    for hc in range(HC):
                        nc.tensor.matmul(
                            q_ps[:],
                            ones_c[:],
                            qT[hc][:, q0 : q0 + qn],
                            start=(hc == 0),
                            stop=(hc == HC - 1),
                        )
                    t1 = rowf.tile([1, qn], F32, tag="t1", bufs=1, name=f"t1{tag}{l}_{q0}")
                    nc.scalar.square(t1[:], s_ps[:])
                    nc.vector.scalar_tensor_tensor(
                        t1[:],
                        t1[:],
                        -1.0 / Hd,
                        q_ps[:],
                        op0=AluOpType.mult,
                        op1=AluOpType.add,
                    )
                    # t1 = sumsq - sum^2/Hd ; rstd = 1/sqrt(t1/Hd + eps)
                    nc.scalar.activation(
                        t1[:], t1[:], AF.Sqrt, scale=1.0 / Hd, bias=eps_sb[0:1, :]
                    )
                    nc.vector.reciprocal(t1[:], t1[:])
                    mr_b = rowb.tile(
                        [1, 2 * qn], BF16, tag="mrb", bufs=1, name=f"mr{tag}{l}_{q0}"
                    )
                    nc.vector.tensor_scalar_mul(mr_b[:, 0:qn], s_ps[:], 1.0 / Hd)
                    nc.vector.tensor_copy(mr_b[:, qn : 2 * qn], t1[:])
                    bpool, btag = (psS, "sc") if q0 >= 512 else (psW, "w512")
                    m_bc = bpool.tile([128, qn], F32, tag=btag, name=f"mbc{tag}{l}_{q0}")
                    r_bc = psW.tile([128, qn], F32, tag="w512", name=f"rbc{tag}{l}_{q0}")
                    nc.tensor.matmul(
                        m_bc[:], ones_r[0:1, 0:128], mr_b[0:1, 0:qn],
                        start=True, stop=True,
                    )
                    nc.tensor.matmul(
                        r_bc[:], ones_r[0:1, 0:128], mr_b[0:1, qn : 2 * qn],
                        start=True, stop=True,
                    )
                    for hc in range(HC):
                        t = scr.tile([128, qn], F32, tag="scr", name=f"sc{tag}{l}_{q0}_{hc}")
                        nc.vector.tensor_sub(t[:], lnin[hc][:, q0 : q0 + qn], m_bc[:])
                        nc.vector.tensor_mul(t[:], t[:], r_bc[:])
                        nc.vector.tensor_scalar(
                            h_m[hc][:, q0 : q0 + qn],
                            t[:],
                            par_t[:, w_off + hc : w_off + hc + 1],
                            par_t[:, b_off + hc : b_off + hc + 1],
                            AluOpType.mult,
                            AluOpType.add,
                        )
                        nc.vector.tensor_copy(
                            h_bf[hc][:, q0 : q0 + qn], h_m[hc][:, q0 : q0 + qn]
                        )

                # O weights loaded once per layer, shared by both q-halves
                wo_t = [
                    wp.tile([128, Hd], BF16, tag="w", name=f"wo{l}_{ic}")
                    for ic in range(HC)
                ]
                for ic in range(HC):
                    nc.sync.dma_start(
                        wo_t[ic][:], wo_d[l, ic * 128 : (ic + 1) * 128, :]
                    )

                for qi, (q0, qn) in enumerate(QS):
                    # ---- attention for this q-half ----
                    _ph(f'attn{qi}', l, nc)
                    for hp in range(HC):
                        ctx = [
                            psC.tile([65, qn], F32, tag="ctx", name=f"ctx{l}_{qi}_{hp}_{p}")
                            for p in range(2)
                        ]
                        for kb in range(KB):
                            half, tb = kb // TPH, kb % TPH
                            sc = psS.tile(
                                [128, 2 * qn], F32, tag="sc", name=f"sc{l}_{qi}_{hp}_{kb}"
                            )
                            for par_i in range(2):
                                b0 = 64 * par_i
                                nc.tensor.matmul(
                                    sc[:, par_i * qn : par_i * qn + qn],
                                    kT[half][hp][b0 : b0 + 64, tb * 128 : (tb + 1) * 128],
                                    qT[hp][b0 : b0 + 64, q0 : q0 + qn],
                                    start=True,
                                    stop=True,
                                    tile_position=(b0, 0),
                                )
                            ex = expp.tile(
                                [128, 2 * qn], BF16, tag="exp", name=f"ex{l}_{qi}_{hp}_{kb}"
                            )
                            nc.scalar.activation(
                                ex[:],
                                sc[:],
                                AF.Exp,
                                bias=mask_sb[:, kb : kb + 1],
                                scale=float(1.0 / np.sqrt(DH)),
                            )
                            for par_i in range(2):
                                head = 2 * hp + par_i
                                nc.tensor.matmul(
                                    ctx[par_i][:, :],
                                    v_sb[kb][:, head * (DH + 1) : (head + 1) * (DH + 1)],
                                    ex[:, par_i * qn : par_i * qn + qn],
                                    start=(kb == 0),
                                    stop=(kb == KB - 1),
                                )
                        # normalize: ctx[0:64] * (1/sumexp) broadcast over partitions
                        for par_i in range(2):
                            rec_b = rowb.tile(
                                [1, qn], BF16, tag="recb", name=f"rb{l}_{qi}_{hp}_{par_i}"
                            )
                            with nc.allow_low_precision("softmax denom in bf16"):
                                nc.vector.reciprocal(rec_b[:], ctx[par_i][64:65, :])
                            bpool2, btag2 = (psW, "w512") if qi == 0 else (psS, "sc")
                            bc = bpool2.tile(
                                [64, qn], F32, tag=btag2, name=f"bc{l}_{qi}_{hp}_{par_i}"
                            )
                            nc.tensor.matmul(
                                bc[:],
                                ones_r[0:1, 0:64],
                                rec_b[0:1, :],
                                start=True,
                                stop=True,
                            )
                            bcs = expp.tile(
                                [64, qn], BF16, tag="bcs", bufs=2, name=f"bcs{l}_{qi}_{hp}_{par_i}"
                            )
                            nc.vector.tensor_copy(bcs[:], bc[:])
                            b0 = 64 * par_i
                            nc.vector.tensor_tensor(
                                ctxT[hp][b0 : b0 + 64, q0 : q0 + qn],
                                ctx[par_i][0:64, :],
                                bcs[:],
                                op=AluOpType.mult,
                            )

                    # ---- O projection + residual -> lnin ----
                    _ph(f'O_proj{qi}', l, nc)
                    for hc in range(HC):
                        wpool, wtag = (psS, "sc") if qi == 1 and hc % 2 else (psW, "w512")
                        ps = wpool.tile([128, qn], F32, tag=wtag, name=f"pso{l}_{qi}_{hc}")
                        for ic in range(HC):
                            nc.tensor.matmul(
                                ps[:],
                                wo_t[ic][:, hc * 128 : (hc + 1) * 128],
                                ctxT[ic][:, q0 : q0 + qn],
                                start=(ic == 0),
                                stop=(ic == HC - 1),
                            )
                        nc.vector.scalar_tensor_tensor(
                            lnin[hc][:, q0 : q0 + qn],
                            ps[:],
                            par_t[:, O_BO + hc : O_BO + hc + 1],
                            h_m[hc][:, q0 : q0 + qn],
                            op0=AluOpType.add,
                            op1=AluOpType.add,
                        )

                    _ph(f'LN1_{qi}', l, nc)
                    layer_norm(q0, qn, O_L1W, O_L1B, "a")

                    # ---- FFN for this q-half ----
                    _ph(f'FFN{qi}', l, nc)
                    for oc in range(FC):
                        if oc % 2 == 0:
                            wi_t = wip.tile(
                                [128, 2 * Hd], BF16, tag="wi", name=f"wi{l}_{qi}_{oc}"
                            )
                            nc.sync.dma_start(wi_t[:], wi_d[l, oc // 2])
                        ob = (oc % 2) * Hd
                        # the trailing half's FFN runs after attention is done,
                        # so the idle score banks double as extra accumulators
                        wpool, wtag = (psS, "sc") if qi == 1 and oc % 2 else (psW, "w512")
                        ps = wpool.tile([128, qn], F32, tag=wtag, name=f"psf{l}_{qi}_{oc}")
                        for ic in range(HC):
                            nc.tensor.matmul(
                                ps[:],
                                wi_t[:, ob + ic * 128 : ob + (ic + 1) * 128],
                                h_bf[ic][:, q0 : q0 + qn],
                                start=(ic == 0),
                                stop=(ic == HC - 1),
                            )
                        nc.scalar.activation(
                            ffT(oc, q0)[:, 0:qn], ps[:], AF.Gelu,
                            bias=bi_t[:, oc : oc + 1],
                        )
                    for hc in range(HC):
                        wf_t = wfp.tile([128, FF], BF16, tag="wf", name=f"wf{l}_{qi}_{hc}")
                        nc.sync.dma_start(wf_t[:], wf_d[l, hc])
                        wpool, wtag = (psS, "sc") if qi == 1 and hc % 2 else (psW, "w512")
                        ps = wpool.tile([128, qn], F32, tag=wtag, name=f"psg{l}_{qi}_{hc}")
                        for fc in range(FC):
                            nc.tensor.matmul(
                                ps[:],
                                wf_t[:, fc * 128 : (fc + 1) * 128],
                                ffT(fc, q0)[:, 0:qn],
                                start=(fc == 0),
                                stop=(fc == FC - 1),
                            )
                        nc.vector.scalar_tensor_tensor(
                            lnin[hc][:, q0 : q0 + qn],
                            ps[:],
                            par_t[:, O_BF + hc : O_BF + hc + 1],
                            h_m[hc][:, q0 : q0 + qn],
                            op0=AluOpType.add,
                            op1=AluOpType.add,
                        )

                    _ph(f'LN2_{qi}', l, nc)
                    layer_norm(q0, qn, O_L2W, O_L2B, "b")

            _ph('output', 99, nc)
            # ------------- output (transpose back to token-major) -------------
            for tb in range(TB):
                ysb = scr.tile([128, Hd], F32, tag="scr", name=f"ysb{tb}")
                for hc in range(HC):
                    tpool, ttag = (psS, "sc") if hc % 2 else (psW, "w512")
                    pst = tpool.tile([128, 128], F32, tag=ttag, name=f"yp{tb}_{hc}")
                    nc.tensor.transpose(
                        pst[:], h_m[hc][:, tb * 128 : (tb + 1) * 128], ident[:]
                    )
                    nc.vector.tensor_copy(ysb[:, hc * 128 : (hc + 1) * 128], pst[:])
                nc.sync.dma_start(y_d[tb * 128 : (tb + 1) * 128, :], ysb[:])

        for fr in reversed(_frees):
            fr()

    nc.compile()
    return nc


# ---------------------------------------------------------------------------
# host-side prep + execution
# ---------------------------------------------------------------------------


def prep_shared_inputs(cfg: Cfg, d: dict) -> dict:
    """Inputs identical on every core (weights)."""
    L, Hd, FF, HC, FC = cfg.L, cfg.Hd, cfg.FF, cfg.HC, cfg.FC

    def colpack(x, n):  # [L, n*128] -> [L, 128, n]
        return np.ascontiguousarray(
            np.asarray(x, np.float32).reshape(L, n, 128).transpose(0, 2, 1)
        )

    par = np.concatenate(
        [
            colpack(d["bq"], HC),
            colpack(d["bk"], HC),
            colpack(d["bo"], HC),
            colpack(d["bf"], HC),
            colpack(d["ln1_w"], HC),
            colpack(d["ln1_b"], HC),
            colpack(d["ln2_w"], HC),
            colpack(d["ln2_b"], HC),
        ],
        axis=2,
    )
    # wi[l, oc, p, ic*128+j] = Wi[l, ic*128+p, oc*128+j], then 4 oc per DMA
    # row-block: wi4[l, g, p, m*Hd + ic*128+j] = wi[l, 4g+m, p, ic*128+j]
    wi_r = np.ascontiguousarray(
        np.asarray(d["Wi"], np.float32)
        .reshape(L, HC, 128, FC, 128)
        .transpose(0, 3, 2, 1, 4)
        .reshape(L, FC // 2, 2, 128, Hd)
        .transpose(0, 1, 3, 2, 4)
        .reshape(L, FC // 2, 128, 2 * Hd)
        .astype(BF)
    )
    # wf[l, oc2, p, fc*128+j] = Wf[l, fc*128+p, oc2*128+j]
    wf_r = np.ascontiguousarray(
        np.asarray(d["Wf"], np.float32)
        .reshape(L, FC, 128, HC, 128)
        .transpose(0, 3, 2, 1, 4)
        .reshape(L, HC, 128, FF)
        .astype(BF)
    )
    return {
        "wq": np.asarray(d["Wq"], np.float32).astype(BF),
        "wk": np.asarray(d["Wk"], np.float32).astype(BF),
        "wv": np.asarray(d["Wv"], np.float32).astype(BF),
        "wo": np.asarray(d["Wo"], np.float32).astype(BF),
        "wi": wi_r,
        "wf": wf_r,
        "par": par,
        "bi": colpack(d["bi"], FC),
        "bv": np.asarray(d["bv"], np.float32).astype(BF)[:, None, :],
        "wemb": np.asarray(d["word_emb"], np.float32).astype(BF),
        "lne": np.concatenate(
            [
                np.asarray(d["ln_e_w"], np.float32).reshape(HC, 128).T,
                np.asarray(d["ln_e_b"], np.float32).reshape(HC, 128).T,
            ],
            axis=1,
        ),
    }


def prep_core_inputs(cfg: Cfg, core: int, d: dict, shared: dict) -> dict:
    TOK, TB, KB = cfg.TOK, cfg.TB, cfg.KB
    b, hh = core // cfg.NHALF, core % cfg.NHALF
    ids = np.asarray(d["input_ids"], np.int32)[b, hh * TOK : (hh + 1) * TOK]
    mask = np.asarray(d["attention_mask"], np.float32)[b, 0, 0, :]
    pos = (
        np.asarray(d["pos_emb"], np.float32)[hh * TOK : (hh + 1) * TOK]
        + np.asarray(d["type_emb"], np.float32)[0][None, :]
    )
    m = dict(shared)
    m["pos"] = np.ascontiguousarray(pos, dtype=np.float32)
    m["ids"] = np.ascontiguousarray(ids.reshape(TB, 128).T)
    m["mask"] = np.ascontiguousarray(mask.reshape(KB, 128).T)
    return m


_CACHE: dict = {}


def kernel(**inputs) -> np.ndarray:
    cfg = Cfg()
    B = inputs["input_ids"].shape[0]
    if "nc" not in _CACHE:
        _CACHE["nc"] = build(cfg)
    nc = _CACHE["nc"]
    shared = prep_shared_inputs(cfg, inputs)
    in_maps = [prep_core_inputs(cfg, c, inputs, shared) for c in range(cfg.n_cores)]
    res = run_bass_kernel_spmd(nc, in_maps, core_ids=list(range(cfg.n_cores)))
    out = np.zeros((B, cfg.S, cfg.Hd), np.float32)
    for c in range(cfg.n_cores):
        b, hh = c // cfg.NHALF, c % cfg.NHALF
        out[b, hh * cfg.TOK : (hh + 1) * cfg.TOK, :] = res.results[c]["y"]
    return out

